# revision 8
# baseline (speedup 1.0000x reference)
"""Trainium2 Bass kernel for nn_CHSHistoryCrossAttentionFusion (8 NeuronCores, SPMD).

Decomposition (hardcoded for B=2, S=4096, L=3, D=1024, N=512, 8 cores):
  - History sequence-sharded: core c owns key positions [c*512, (c+1)*512) of
    each batch; it computes its chunk of fused/K/V from its x chunk.
  - Queries sharded 8-way for the Q path (64 batch-0 + 64 batch-1 queries per
    core); an AllGather replicates Q (bf16, small) so every core scores all
    1024 queries against its own K/V chunk.
  - Flash-style partial softmax per chunk WITHOUT max subtraction (Q/K are
    RMS-normalized so scores are bounded); causal mask applied additively
    before exp; exp carries a constant -ln(256) prescale so the (o,l)
    partials fit fp16.  Partials combine via two fp16 ReduceScatter-adds
    (one per batch), which also re-shard queries for the epilogue.
  - x is supplied host-side as 9 pre-transposed [3072,128] strips (8 history
    tiles + 1 gathered query tile), so the fc matmul needs no on-chip
    transposes; f32->bf16 conversion of x and most square weights happens
    inside SWDGE cast-DMAs (gpsimd queue); wfc/wk/wv are staged f32 on the
    two HWDGE queues and cast on the vector engine.  The fc matmul runs
    kk-outer over groups of 3 token tiles so the PE consumes weight slices
    at DMA arrival rate; the Q AllGather triggers as soon as the query tile
    clears the fc+Wq projections (~50us in).
  - All matmuls bf16 (fp32 accumulate); f32 in/out.
Host-side work is layout/indexing only.
"""

import math
import os

import numpy as np

try:
    import ml_dtypes
except ImportError:  # pragma: no cover
    ml_dtypes = None

import concourse.bacc as bacc
import concourse.mybir as mybir
import concourse.tile as tile
import concourse.tile_utils as tile_utils
from concourse.bass_utils import run_bass_kernel_spmd

# cayman has 208 KiB/partition usable; the default constant leaves 16 KiB idle
tile_utils.max_sbuf_usage = 208 * 1024

F32 = mybir.dt.float32
F16 = mybir.dt.float16
BF16 = mybir.dt.bfloat16
AF = mybir.ActivationFunctionType
OP = mybir.AluOpType

B, S, L, D = 2, 4096, 3, 1024
N = 512
NC = 8
CH = S // NC              # 512 keys per batch per core
LD = L * D                # 3072
QT = B * N                # 1024 global queries
QPC = QT // NC            # 128 queries per core (64 per batch)
NKK = LD // 128           # 24 contraction slices over 3072
NJ = D // 128             # 8 contraction slices over 1024
NT = 9                    # 8 history tiles + 1 query tile
RMS_EPS = 1e-6
SCALE = D ** -0.5
MASK_NEG = -1.0e6
EXP_BIAS = -math.log(256.0)

_CACHE = {}


def _build(apply_norm_weights: bool):
    nc = bacc.Bacc("TRN2", target_bir_lowering=False, num_devices=NC)

    # ---------------- I/O ----------------
    # x3: strip 0 = gathered query columns (x^T), strips 1..8 = history tiles
    # t0..t7 (batch-major) transposed to [3072, 128].
    x3 = nc.dram_tensor("x3", [NT * 128, NKK * 128], F32, kind="ExternalInput")
    wfc = nc.dram_tensor("wfc", [LD, D], F32, kind="ExternalInput")
    wq = nc.dram_tensor("wq", [D, D], F32, kind="ExternalInput")
    wk = nc.dram_tensor("wk", [D, D], F32, kind="ExternalInput")
    wv = nc.dram_tensor("wv", [D, D], F32, kind="ExternalInput")
    wo = nc.dram_tensor("wo", [D, D], F32, kind="ExternalInput")
    pet = nc.dram_tensor("pet", [D, CH], F32, kind="ExternalInput")
    peq = nc.dram_tensor("peq", [QPC, D], F32, kind="ExternalInput")
    thr = nc.dram_tensor("thr", [128, NC], F32, kind="ExternalInput")
    iota = nc.dram_tensor("iota", [128, CH], F32, kind="ExternalInput")
    ident = nc.dram_tensor("ident", [128, 128], BF16, kind="ExternalInput")
    if apply_norm_weights:
        whn = nc.dram_tensor("whn", [128, D], F32, kind="ExternalInput")
        wqn = nc.dram_tensor("wqn", [128, D], F32, kind="ExternalInput")
        wkn = nc.dram_tensor("wkn", [128, D], F32, kind="ExternalInput")
        won = nc.dram_tensor("won", [128, D], F32, kind="ExternalInput")
    out = nc.dram_tensor("out", [QPC, D], F32, kind="ExternalOutput")

    def sliced(t):
        """DRAM [rows,cols] -> 3d AP [128, rows/128, cols] (partition-major)."""
        return t.ap().rearrange("(s p) c -> p s c", p=128)

    with tile.TileContext(nc) as tc:
        with (
            tc.tile_pool(name="dram", bufs=1, space="DRAM") as dram,
            tc.tile_pool(name="const", bufs=1) as constp,
            tc.tile_pool(name="stat", bufs=6) as stat,
            tc.tile_pool(name="wpool", bufs=1) as wpool,
            tc.tile_pool(name="strip", bufs=5) as stripp,
            tc.tile_pool(name="base", bufs=1) as base,
            tc.tile_pool(name="scr_bf", bufs=3) as scr_bf,
            tc.tile_pool(name="scr_f", bufs=3) as scr_f,
            tc.tile_pool(name="mmps", bufs=4, space="PSUM") as mmps,
        ):
            # collective bounce buffers
            ag_in = dram.tile([QPC, D], BF16)
            ag_out = dram.tile([QT, D], BF16, addr_space="Shared")
            rs_inA = dram.tile([N, D + 1], F16)
            rs_outA = dram.tile([N // NC, D + 1], F16)
            rs_inB = dram.tile([N, D + 1], F16)
            rs_outB = dram.tile([N // NC, D + 1], F16)

            # small constants (sync queue, land in the first ~2us)
            iota_sb = constp.tile([128, CH], F32)
            nc.sync.dma_start(iota_sb[:], iota[:])
            thr_sb = constp.tile([128, NC], F32)
            nc.sync.dma_start(thr_sb[:], thr[:])
            eps_sb = constp.tile([128, 1], F32)
            nc.vector.memset(eps_sb[:], RMS_EPS)
            ebias_sb = constp.tile([128, 1], F32)
            nc.vector.memset(ebias_sb[:], EXP_BIAS)
            if apply_norm_weights:
                whn_sb = constp.tile([128, D], F32)
                nc.sync.dma_start(whn_sb[:], whn[:])
                wqn_sb = constp.tile([128, D], F32)
                nc.sync.dma_start(wqn_sb[:], wqn[:])
                wkn_sb = constp.tile([128, D], F32)
                nc.sync.dma_start(wkn_sb[:], wkn[:])
                won_sb = constp.tile([128, D], F32)
                nc.sync.dma_start(won_sb[:], won[:])

            # ---------- bulk loads (part 1) ----------
            # SWDGE (gpsimd) casts f32->bf16 inline.  Early queue order:
            # query strip, wq, peq, strips t0..t3, pet.  Strips t4..t7 are
            # emitted AFTER the AllGather trigger so they don't delay it on
            # the in-order gpsimd queue; wo is emitted last (shares wk's
            # SBUF slot, WAR-resolved after batch-1 K projections).
            strips = [None] * 8

            def load_strip(t, name):
                st = stripp.tile([128, NKK * 128], BF16, tag="strip",
                                 name=name)
                idx = 0 if t is None else 1 + t
                nc.gpsimd.dma_start(
                    st[:], x3.ap()[idx * 128:(idx + 1) * 128, :])
                return st

            strip_q = load_strip(None, "strip_q")
            wq_sb = wpool.tile([128, NJ * D], BF16, tag="wqv", name="wq_sb")
            nc.gpsimd.dma_start(
                wq_sb[:].rearrange("p (s c) -> p s c", s=NJ), sliced(wq))
            peq_bf = wpool.tile([QPC, D], BF16)
            nc.gpsimd.dma_start(peq_bf[:], peq.ap())
            for t in range(2):
                strips[t] = load_strip(t, f"strip{t}")
            pet_bf = wpool.tile([128, NJ * CH], BF16)     # [d_lo, j*512+tc]
            nc.gpsimd.dma_start(
                pet_bf[:].rearrange("p (s c) -> p s c", s=NJ), sliced(pet))
            pet_v = pet_bf[:].rearrange("p (j t) -> p j t", j=NJ)
            wk_sb = wpool.tile([128, NJ * D], BF16, tag="wko", name="wk_sb")
            nc.gpsimd.dma_start(
                wk_sb[:].rearrange("p (s c) -> p s c", s=NJ), sliced(wk))
            for t in range(2, 4):
                strips[t] = load_strip(t, f"strip{t}")

            # wfc: f32 slices alternating across the two HWDGE queues,
            # cast on vector.  Stage tiles share the scr_f "sqscr" tag
            # (reused as rms scratch later).
            wfc_bf = wpool.tile([128, NKK * D], BF16)
            for s_ in range(NKK):
                stg = scr_f.tile([128, D], F32, tag="sqscr", name=f"wfcst{s_}")
                eng = nc.sync if s_ % 2 == 0 else nc.scalar
                eng.dma_start(stg[:], wfc.ap()[s_ * 128:(s_ + 1) * 128, :])
                nc.vector.tensor_copy(wfc_bf[:, s_ * D:(s_ + 1) * D], stg[:])

            # persistent activations
            fusedT_b = [base.tile([128, NJ * CH], BF16, name=f"fusedT{b}")
                        for b in range(B)]
            fusedT_bv = [fT[:].rearrange("p (j t) -> p j t", j=NJ)
                         for fT in fusedT_b]
            qs_f32 = base.tile([QPC, D], F32)
            kT = base.tile([128, NJ * CH], BF16, name="kT")
            kT_v = kT[:].rearrange("p (j t) -> p j t", j=NJ)
            v_b = base.tile([128, 4 * D], BF16, name="v_b")

            def rms_stats(src_ap):
                sq = scr_f.tile([128, D], F32, tag="sqscr")
                ssq = stat.tile([128, 1], F32, tag="ssq")
                nc.scalar.activation(sq[:], src_ap, AF.Square, accum_out=ssq[:])
                std = stat.tile([128, 1], F32, tag="std")
                nc.scalar.activation(std[:], ssq[:], AF.Sqrt, scale=1.0 / D,
                                     bias=eps_sb[:])
                rstd = stat.tile([128, 1], F32, tag="rstd")
                nc.vector.reciprocal(rstd[:], std[:])
                return rstd

            def dma_transpose(dst_ap_3d, src_ap):
                """Xbar-transpose [128, n*128] bf16 into dst 3d view [128,n,128]."""
                nc.scalar.dma_start(dst_ap_3d, src_ap, transpose=True)

            # ---------------- phase 1: fc matmul, kk-outer groups ----------
            fps_tiles = {}

            def fc_group(tiles):
                """tiles: list of (key, strip_tile).  key: 't8' or 0..7."""
                for key, _ in tiles:
                    fps_tiles[key] = mmps.tile([128, D], F32, tag="mm",
                                               name=f"fps{key}")
                for kk in range(NKK):
                    for key, st in tiles:
                        fps = fps_tiles[key]
                        for h in range(2):
                            nc.tensor.matmul(
                                fps[:, h * 512:(h + 1) * 512],
                                st[:, kk * 128:(kk + 1) * 128],
                                wfc_bf[:, kk * D + h * 512: kk * D + h * 512 + 512],
                                start=(kk == 0),
                                stop=(kk == NKK - 1),
                            )

            def fused_epilogue(t):
                """History tile t: rms-normalize + transpose into fusedT."""
                fps = fps_tiles[t]
                rstd = rms_stats(fps[:])
                fb = scr_bf.tile([128, D], BF16, tag="tmb")
                nc.vector.tensor_scalar(fb[:], fps[:], rstd[:], None, OP.mult)
                if apply_norm_weights:
                    nc.vector.tensor_tensor(fb[:], fb[:], whn_sb[:], op=OP.mult)
                bb, tl = divmod(t, 4)
                dma_transpose(
                    fusedT_bv[bb][:, :, tl * 128:(tl + 1) * 128], fb[:])

            def q_epilogue():
                fps = fps_tiles["t8"]
                rstd = rms_stats(fps[:])
                nc.vector.tensor_scalar(qs_f32[:], fps[:], rstd[:], None,
                                        OP.mult)
                if apply_norm_weights:
                    nc.vector.tensor_tensor(qs_f32[:], qs_f32[:], whn_sb[:],
                                            op=OP.mult)
                qhb = scr_bf.tile([128, D], BF16, tag="tmb")
                nc.vector.tensor_scalar(qhb[:], fps[:], rstd[:], None, OP.mult)
                if apply_norm_weights:
                    nc.vector.tensor_tensor(qhb[:], qhb[:], whn_sb[:],
                                            op=OP.mult)
                nc.vector.tensor_add(qhb[:], qhb[:], peq_bf[:])
                qht = scr_bf.tile([128, D], BF16, tag="tmb")
                dma_transpose(qht[:].rearrange("p (j c) -> p j c", j=NJ),
                              qhb[:])
                qps = mmps.tile([128, D], F32, tag="mm", name="qps")
                for j in range(NJ):
                    for h in range(2):
                        nc.tensor.matmul(
                            qps[:, h * 512:(h + 1) * 512],
                            qht[:, j * 128:(j + 1) * 128],
                            wq_sb[:, j * D + h * 512: j * D + h * 512 + 512],
                            start=(j == 0),
                            stop=(j == NJ - 1),
                        )
                qrstd = rms_stats(qps[:])
                qb = scr_bf.tile([128, D], BF16, tag="tmb")
                nc.vector.tensor_scalar(qb[:], qps[:], qrstd[:], None, OP.mult)
                if apply_norm_weights:
                    nc.vector.tensor_tensor(qb[:], qb[:], wqn_sb[:], op=OP.mult)
                nc.sync.dma_start(ag_in[:], qb[:])
                nc.gpsimd.collective_compute(
                    "AllGather", OP.bypass,
                    replica_groups=[list(range(NC))],
                    ins=[ag_in.opt()],
                    outs=[ag_out.opt()],
                )

            # -------- phase 2 helpers: K^T, V per tile; attention ----------
            def k_tile(bb, tl):
                khb = scr_bf.tile([128, NJ * 128], BF16, tag="khb", bufs=2)
                nc.vector.tensor_add(
                    khb[:].rearrange("p (j x) -> p j x", j=NJ),
                    fusedT_bv[bb][:, :, tl * 128:(tl + 1) * 128],
                    pet_v[:, :, tl * 128:(tl + 1) * 128],
                )
                kps = mmps.tile([128, D], F32, tag="mm")
                for j in range(NJ):
                    for h in range(2):
                        nc.tensor.matmul(
                            kps[:, h * 512:(h + 1) * 512],
                            khb[:, j * 128:(j + 1) * 128],
                            wk_sb[:, j * D + h * 512: j * D + h * 512 + 512],
                            start=(j == 0),
                            stop=(j == NJ - 1),
                        )
                krstd = rms_stats(kps[:])
                kb = scr_bf.tile([128, D], BF16, tag="tmb")
                nc.vector.tensor_scalar(kb[:], kps[:], krstd[:], None, OP.mult)
                if apply_norm_weights:
                    nc.vector.tensor_tensor(kb[:], kb[:], wkn_sb[:], op=OP.mult)
                dma_transpose(kT_v[:, :, tl * 128:(tl + 1) * 128], kb[:])

            def v_tile(bb, tl):
                vps = mmps.tile([128, D], F32, tag="mm")
                for j in range(NJ):
                    for h in range(2):
                        nc.tensor.matmul(
                            vps[:, h * 512:(h + 1) * 512],
                            fusedT_bv[bb][:, j:j + 1,
                                          tl * 128:(tl + 1) * 128]
                            .rearrange("p j x -> p (j x)"),
                            wv_sb[:, j * D + h * 512: j * D + h * 512 + 512],
                            start=(j == 0),
                            stop=(j == NJ - 1),
                        )
                nc.scalar.copy(v_b[:, tl * D:(tl + 1) * D], vps[:])

            def make_masks(lo):
                mk = scr_bf.tile([128, 4 * CH], BF16, tag="mask", bufs=1,
                                 name=f"mask{lo}")
                for u in range(4):
                    nc.vector.tensor_scalar(
                        mk[:, u * CH:(u + 1) * CH], iota_sb[:],
                        thr_sb[:, lo + u:lo + u + 1], MASK_NEG,
                        OP.is_gt, OP.mult)
                return mk

            def q_tile_T(i):
                """Gather q-tile i from ag_out and transpose into a slot."""
                b, k2 = divmod(i % 4, 1)  # placeholder; real mapping below
                bb, k2 = divmod(i, 4)
                qg = scr_bf.tile([128, D], BF16, tag="tmb")
                r0 = (2 * k2) * 128 + bb * 64
                r1 = (2 * k2 + 1) * 128 + bb * 64
                nc.sync.dma_start(qg[0:64, :], ag_out[r0:r0 + 64, :])
                nc.sync.dma_start(qg[64:128, :], ag_out[r1:r1 + 64, :])
                qTt = scr_bf.tile([128, NJ * 128], BF16, tag="qTt", bufs=3,
                                  name=f"qT{i}")
                dma_transpose(qTt[:].rearrange("p (j x) -> p j x", j=NJ),
                              qg[:])
                return qTt

            def attn_tile(i, qTt, mask_ap, rs_buf, row0):
                sps = mmps.tile([128, CH], F32, tag="mm")
                for j in range(NJ):
                    nc.tensor.matmul(
                        sps[:],
                        qTt[:, j * 128:(j + 1) * 128],
                        kT[:, j * CH:(j + 1) * CH],
                        start=(j == 0),
                        stop=(j == NJ - 1),
                    )
                sm = scr_f.tile([128, CH], F32, tag="mb", bufs=2)
                nc.vector.tensor_add(sm[:], sps[:], mask_ap)
                o_sb = scr_f.tile([128, D + 1], F16, tag="osb", bufs=3)
                lacc = stat.tile([128, 1], F32, tag="lacc")
                probs = scr_bf.tile([128, CH], BF16, tag="probs", bufs=2)
                nc.scalar.activation(probs[:], sm[:], AF.Exp, scale=SCALE,
                                     bias=ebias_sb[:], accum_out=lacc[:])
                nc.vector.tensor_copy(o_sb[:, D:D + 1], lacc[:])
                pT = scr_bf.tile([128, 512], BF16, tag="pT", bufs=2)
                dma_transpose(pT[:].rearrange("p (u x) -> p u x", u=4),
                              probs[:])
                ops_ = mmps.tile([128, D], F32, tag="mm")
                for u in range(4):
                    for h in range(2):
                        nc.tensor.matmul(
                            ops_[:, h * 512:(h + 1) * 512],
                            pT[:, u * 128:(u + 1) * 128],
                            v_b[:, u * D + h * 512: u * D + h * 512 + 512],
                            start=(u == 0),
                            stop=(u == 3),
                        )
                nc.vector.tensor_copy(o_sb[:, 0:D], ops_[:])
                nc.scalar.dma_start(rs_buf[row0:row0 + 128, :], o_sb[:])

            # ---------------- emission schedule ----------------
            fc_group([("t8", strip_q), (0, strips[0]), (1, strips[1])])
            q_epilogue()
            # wv + strips t4..t7 queue behind the AllGather trigger on the
            # in-order gpsimd queue (their WARs/dispatch don't delay it)
            wv_sb = wpool.tile([128, NJ * D], BF16, tag="wqv", name="wv_sb")
            nc.gpsimd.dma_start(
                wv_sb[:].rearrange("p (s c) -> p s c", s=NJ), sliced(wv))
            for t in range(4, 8):
                strips[t] = load_strip(t, f"strip{t}")
            fused_epilogue(0)
            fused_epilogue(1)
            maskA = make_masks(0)
            k_tile(0, 0)
            v_tile(0, 0)
            k_tile(0, 1)
            v_tile(0, 1)
            fc_group([(2, strips[2]), (3, strips[3])])
            fused_epilogue(2)
            fused_epilogue(3)
            k_tile(0, 2)
            v_tile(0, 2)
            k_tile(0, 3)
            v_tile(0, 3)

            fc_group([(4, strips[4]), (5, strips[5]), (6, strips[6])])
            fused_epilogue(4)
            fused_epilogue(5)
            fused_epilogue(6)
            fc_group([(7, strips[7])])
            fused_epilogue(7)

            for i in range(4):
                qTt = q_tile_T(i)
                attn_tile(i, qTt, maskA[:, (i % 4) * CH:(i % 4 + 1) * CH],
                          rs_inA, i * 128)
            nc.gpsimd.collective_compute(
                "ReduceScatter", OP.add,
                replica_groups=[list(range(NC))],
                ins=[rs_inA.opt()],
                outs=[rs_outA.opt()],
            )

            maskB = make_masks(4)
            for tl in range(4):
                k_tile(1, tl)
                v_tile(1, tl)
            for i in range(4, 8):
                qTt = q_tile_T(i)
                attn_tile(i, qTt, maskB[:, (i % 4) * CH:(i % 4 + 1) * CH],
                          rs_inB, (i - 4) * 128)
            nc.gpsimd.collective_compute(
                "ReduceScatter", OP.add,
                replica_groups=[list(range(NC))],
                ins=[rs_inB.opt()],
                outs=[rs_outB.opt()],
            )

            # wo load (SWDGE, reuses wk's slot) — emitted after the RS_B
            # trigger so its WAR wait doesn't block the collectives on the
            # in-order gpsimd queue.
            wo_sb = wpool.tile([128, NJ * D], BF16, tag="wko", name="wo_sb")
            nc.gpsimd.dma_start(
                wo_sb[:].rearrange("p (s c) -> p s c", s=NJ), sliced(wo))

            # ---------------- epilogue for own 128 queries --------------
            fo = scr_f.tile([QPC, D + 1], F16, tag="fo", bufs=1)
            nc.sync.dma_start(fo[0:64, :], rs_outA[:])
            nc.sync.dma_start(fo[64:128, :], rs_outB[:])
            linv = stat.tile([128, 1], F32, tag="linv")
            nc.vector.reciprocal(linv[:], fo[:, D:D + 1])
            ao = scr_bf.tile([128, D], BF16, tag="tmb")
            nc.vector.tensor_scalar(ao[:], fo[:, 0:D], linv[:], None, OP.mult)
            aoT = scr_bf.tile([128, D], BF16, tag="tmb")
            dma_transpose(aoT[:].rearrange("p (j c) -> p j c", j=NJ), ao[:])
            zps = mmps.tile([128, D], F32, tag="mm")
            for j in range(NJ):
                for h in range(2):
                    nc.tensor.matmul(
                        zps[:, h * 512:(h + 1) * 512],
                        aoT[:, j * 128:(j + 1) * 128],
                        wo_sb[:, j * D + h * 512: j * D + h * 512 + 512],
                        start=(j == 0),
                        stop=(j == NJ - 1),
                    )
            hh = scr_f.tile([128, D], F32, tag="sqscr")
            nc.vector.tensor_add(hh[:], qs_f32[:], zps[:])
            orstd = rms_stats(hh[:])
            yv = scr_f.tile([128, D], F32, tag="sqscr")
            nc.vector.tensor_scalar(yv[:], hh[:], orstd[:], None, OP.mult)
            if apply_norm_weights:
                nc.vector.tensor_tensor(yv[:], yv[:], won_sb[:], op=OP.mult)
            nc.sync.dma_start(out[:], yv[:])

    nc.compile()
    return nc


def _pe_table():
    half = D // 2
    inv_freq = np.exp(np.arange(half, dtype=np.float32)
                      * (-math.log(10000.0) / half))
    ang = np.arange(S, dtype=np.float32)[:, None] * inv_freq
    return np.concatenate([np.sin(ang), np.cos(ang)], axis=-1).astype(np.float32)


def _core_gidx(c):
    """Global query indices owned by core c (64 batch-0 then 64 batch-1)."""
    h = QPC // 2
    return np.concatenate([np.arange(c * h, (c + 1) * h),
                           N + np.arange(c * h, (c + 1) * h)])


def make_in_maps(np_inputs, apply_w=False):
    hid = np.asarray(np_inputs["hidden_states"], np.float32)
    pos = np.asarray(np_inputs["context_positions"])
    Wfc = np.ascontiguousarray(np.asarray(np_inputs["W_fc"], np.float32))
    Wq = np.ascontiguousarray(np.asarray(np_inputs["Wq"], np.float32))
    Wk = np.ascontiguousarray(np.asarray(np_inputs["Wk"], np.float32))
    Wv = np.ascontiguousarray(np.asarray(np_inputs["Wv"], np.float32))
    Wo = np.ascontiguousarray(np.asarray(np_inputs["Wo"], np.float32))

    x = hid.reshape(B, S, LD)
    p = np.clip(pos.astype(np.int64), 0, S - 1)
    p_flat = p.reshape(QT)
    PE = _pe_table()

    iota_np = np.tile(np.arange(CH, dtype=np.float32), (128, 1))
    ident_np = np.eye(128, dtype=np.float32).astype(ml_dtypes.bfloat16)

    in_maps = []
    for c in range(NC):
        sl = slice(c * CH, (c + 1) * CH)
        gidx = _core_gidx(c)
        # strip layout [128 part, kk, 128 tok]: elem (p,kk,c) = x^T[kk*128+p, c]
        x3 = np.empty((NT, 128, NKK, 128), np.float32)
        xq_cols = x[gidx // N, p_flat[gidx]].T          # [3072, 128]
        x3[0] = xq_cols.reshape(NKK, 128, 128).transpose(1, 0, 2)
        for t in range(8):
            bb, tl = divmod(t, 4)
            r0 = c * CH + tl * 128
            x3[1 + t] = (x[bb, r0:r0 + 128, :].T
                         .reshape(NKK, 128, 128).transpose(1, 0, 2))
        peq_a = np.ascontiguousarray(PE[p_flat[gidx]])
        pet_a = np.ascontiguousarray(PE[sl].T)
        thr_a = np.ascontiguousarray(
            (p_flat.astype(np.float32) - c * CH).reshape(NC, 128).T)
        m = {
            "x3": x3.reshape(NT * 128, NKK * 128),
            "wfc": Wfc, "wq": Wq, "wk": Wk, "wv": Wv, "wo": Wo,
            "pet": pet_a, "peq": peq_a, "thr": thr_a,
            "iota": iota_np, "ident": ident_np,
        }
        if apply_w:
            m["whn"] = np.tile(np.asarray(np_inputs["w_hidden_norm"], np.float32), (128, 1))
            m["wqn"] = np.tile(np.asarray(np_inputs["w_q_norm"], np.float32), (128, 1))
            m["wkn"] = np.tile(np.asarray(np_inputs["w_k_norm"], np.float32), (128, 1))
            m["won"] = np.tile(np.asarray(np_inputs["w_out_norm"], np.float32), (128, 1))
        in_maps.append(m)
    return in_maps


def assemble_out(results):
    y = np.zeros((QT, D), np.float32)
    for c in range(NC):
        y[_core_gidx(c)] = results[c]["out"]
    return y.reshape(B, N, D)


def kernel(**inputs) -> np.ndarray:
    w_h = np.asarray(inputs["w_hidden_norm"], np.float32)
    w_q = np.asarray(inputs["w_q_norm"], np.float32)
    w_k = np.asarray(inputs["w_k_norm"], np.float32)
    w_o = np.asarray(inputs["w_out_norm"], np.float32)
    apply_w = not (np.all(w_h == 1) and np.all(w_q == 1)
                   and np.all(w_k == 1) and np.all(w_o == 1))

    key = ("nc", apply_w)
    if key not in _CACHE:
        _CACHE[key] = _build(apply_w)
    nc = _CACHE[key]

    in_maps = make_in_maps(inputs, apply_w)

    trace = os.environ.get("KERNEL_TRACE", "0") == "1"
    if trace:
        try:
            import axon_prof
            axon_prof.install()
        except Exception:
            trace = False
    res = run_bass_kernel_spmd(nc, in_maps, list(range(NC)), trace=trace)
    global LAST_EXEC_NS
    LAST_EXEC_NS = res.exec_time_ns

    return assemble_out(res.results).astype(np.float32)


LAST_EXEC_NS = None


# revision 11
# speedup vs baseline: 1.2941x; 1.2941x over previous
"""Trainium2 Bass kernel for nn_CHSHistoryCrossAttentionFusion (8 NeuronCores, SPMD).

Decomposition (hardcoded for B=2, S=4096, L=3, D=1024, N=512, 8 cores):
  - History sequence-sharded: core c owns key positions [c*512, (c+1)*512) of
    each batch; it computes its chunk of fused/K/V from its x chunk.
  - Queries sharded 8-way for the Q path (64 batch-0 + 64 batch-1 queries per
    core); an AllGather replicates Q (bf16, small) so every core scores all
    1024 queries against its own K/V chunk.
  - Flash-style partial softmax per chunk WITHOUT max subtraction (Q/K are
    RMS-normalized so scores are bounded); causal mask applied additively
    before exp; exp carries a constant -ln(256) prescale so the (o,l)
    partials fit fp16.  Partials combine via two fp16 ReduceScatter-adds
    (one per batch), which also re-shard queries for the epilogue.
  - x is supplied host-side as 9 pre-transposed, partition-reblocked
    [128, 24*128] strips (8 history tiles + 1 gathered query tile) so the
    fc matmul needs no on-chip transposes and strip DMAs run 12KB-contiguous.
  - f32->bf16 conversion of x / wq / pet / peq / wo happens inside SWDGE
    cast-DMAs (gpsimd queue); wfc alternates f32 slices across the two
    HWDGE queues (+vector casts); wk/wv are staged behind wfc on the
    scalar queue.  The fc runs kk-outer over {query,t0,t1} while the wfc
    stream arrives (PE paced to DMA with sub-throttle-window gaps), then
    tile-major full-rate for t2..t7 (the in-order PE queue never waits on
    a not-yet-arrived strip).
  - All matmuls bf16 (fp32 accumulate); f32 in/out.
Host-side work is layout/indexing only.
"""

import math
import os

import numpy as np

try:
    import ml_dtypes
except ImportError:  # pragma: no cover
    ml_dtypes = None

import concourse.bacc as bacc
import concourse.mybir as mybir
import concourse.tile as tile
import concourse.tile_utils as tile_utils
from concourse.bass_utils import run_bass_kernel_spmd

# cayman has 208 KiB/partition usable; the default constant leaves 16 KiB idle
tile_utils.max_sbuf_usage = 208 * 1024

F32 = mybir.dt.float32
F16 = mybir.dt.float16
BF16 = mybir.dt.bfloat16
AF = mybir.ActivationFunctionType
OP = mybir.AluOpType

B, S, L, D = 2, 4096, 3, 1024
N = 512
NC = 8
CH = S // NC              # 512 keys per batch per core
LD = L * D                # 3072
QT = B * N                # 1024 global queries
QPC = QT // NC            # 128 queries per core (64 per batch)
NKK = LD // 128           # 24 contraction slices over 3072
NJ = D // 128             # 8 contraction slices over 1024
NT = 9                    # 8 history tiles + 1 query tile
RMS_EPS = 1e-6
SCALE = D ** -0.5
MASK_NEG = -1.0e6
EXP_BIAS = -math.log(256.0)

_CACHE = {}


def _build(apply_norm_weights: bool):
    nc = bacc.Bacc("TRN2", target_bir_lowering=False, num_devices=NC)

    # ---------------- I/O ----------------
    x3 = nc.dram_tensor("x3", [NT * 128, NKK * 128], F32, kind="ExternalInput")
    wfc = nc.dram_tensor("wfc", [LD, D], F32, kind="ExternalInput")
    wq = nc.dram_tensor("wq", [D, D], F32, kind="ExternalInput")
    wk = nc.dram_tensor("wk", [D, D], F32, kind="ExternalInput")
    wv = nc.dram_tensor("wv", [D, D], F32, kind="ExternalInput")
    wo = nc.dram_tensor("wo", [D, D], F32, kind="ExternalInput")
    pet = nc.dram_tensor("pet", [D, CH], F32, kind="ExternalInput")
    peq = nc.dram_tensor("peq", [QPC, D], F32, kind="ExternalInput")
    thr = nc.dram_tensor("thr", [128, NC], F32, kind="ExternalInput")
    iota = nc.dram_tensor("iota", [128, CH], F32, kind="ExternalInput")
    ident = nc.dram_tensor("ident", [128, 128], BF16, kind="ExternalInput")
    if apply_norm_weights:
        whn = nc.dram_tensor("whn", [128, D], F32, kind="ExternalInput")
        wqn = nc.dram_tensor("wqn", [128, D], F32, kind="ExternalInput")
        wkn = nc.dram_tensor("wkn", [128, D], F32, kind="ExternalInput")
        won = nc.dram_tensor("won", [128, D], F32, kind="ExternalInput")
    out = nc.dram_tensor("out", [QPC, D], F32, kind="ExternalOutput")

    def sliced(t):
        """DRAM [rows,cols] -> 3d AP [128, rows/128, cols] (partition-major)."""
        return t.ap().rearrange("(s p) c -> p s c", p=128)

    with tile.TileContext(nc) as tc:
        with (
            tc.tile_pool(name="dram", bufs=1, space="DRAM") as dram,
            tc.tile_pool(name="const", bufs=1) as constp,
            tc.tile_pool(name="stat", bufs=6) as stat,
            tc.tile_pool(name="wpool", bufs=1) as wpool,
            tc.tile_pool(name="strip", bufs=4) as stripp,
            tc.tile_pool(name="base", bufs=1) as base,
            tc.tile_pool(name="scr_bf", bufs=3) as scr_bf,
            tc.tile_pool(name="scr_f", bufs=3) as scr_f,
            tc.tile_pool(name="mmps", bufs=3, space="PSUM") as mmps,
            tc.tile_pool(name="trps", bufs=2, space="PSUM") as trps,
        ):
            # collective bounce buffers
            ag_in = dram.tile([QPC, D], BF16)
            ag_out = dram.tile([QT, D], BF16, addr_space="Shared")
            rs_inA = dram.tile([N, D + 1], F16)
            rs_outA = dram.tile([N // NC, D + 1], F16)
            rs_inB = dram.tile([N, D + 1], F16)
            rs_outB = dram.tile([N // NC, D + 1], F16)

            # small constants (sync queue, land in the first ~2us)
            id_sb = constp.tile([128, 128], BF16)
            nc.sync.dma_start(id_sb[:], ident[:])
            iota_sb = constp.tile([128, CH], F32)
            nc.sync.dma_start(iota_sb[:], iota[:])
            thr_sb = constp.tile([128, NC], F32)
            nc.sync.dma_start(thr_sb[:], thr[:])
            eps_sb = constp.tile([128, 1], F32)
            nc.vector.memset(eps_sb[:], RMS_EPS)
            ebias_sb = constp.tile([128, 1], F32)
            nc.vector.memset(ebias_sb[:], EXP_BIAS)
            if apply_norm_weights:
                whn_sb = constp.tile([128, D], F32)
                nc.sync.dma_start(whn_sb[:], whn[:])
                wqn_sb = constp.tile([128, D], F32)
                nc.sync.dma_start(wqn_sb[:], wqn[:])
                wkn_sb = constp.tile([128, D], F32)
                nc.sync.dma_start(wkn_sb[:], wkn[:])
                won_sb = constp.tile([128, D], F32)
                nc.sync.dma_start(won_sb[:], won[:])

            # ---------- bulk loads (part 1) ----------
            strips = [None] * 8

            def load_strip(t, name):
                st = stripp.tile([128, NKK * 128], BF16, tag="strip",
                                 name=name)
                idx = 0 if t is None else 1 + t
                nc.gpsimd.dma_start(
                    st[:], x3.ap()[idx * 128:(idx + 1) * 128, :])
                return st

            strip_q = load_strip(None, "strip_q")
            for t in range(3):
                strips[t] = load_strip(t, f"strip{t}")
            wq_sb = wpool.tile([128, NJ * D], BF16, tag="wqv", name="wq_sb")
            nc.gpsimd.dma_start(
                wq_sb[:].rearrange("p (s c) -> p s c", s=NJ), sliced(wq))
            peq_bf = wpool.tile([QPC, D], BF16)
            nc.gpsimd.dma_start(peq_bf[:], peq.ap())
            pet_bf = wpool.tile([128, NJ * CH], BF16)     # [d_lo, j*512+tc]
            nc.gpsimd.dma_start(
                pet_bf[:].rearrange("p (s c) -> p s c", s=NJ), sliced(pet))
            pet_v = pet_bf[:].rearrange("p (j t) -> p j t", j=NJ)

            # wfc: f32 slices alternating across the two HWDGE queues,
            # cast on vector.  Stage tiles share the scr_f "sqscr" tag.
            wfc_bf = wpool.tile([128, NKK * D], BF16)
            for s_ in range(NKK):
                stg = scr_f.tile([128, D], F32, tag="sqscr", name=f"wfcst{s_}")
                eng = nc.sync if s_ % 2 == 0 else nc.scalar
                eng.dma_start(stg[:], wfc.ap()[s_ * 128:(s_ + 1) * 128, :])
                nc.vector.tensor_copy(wfc_bf[:, s_ * D:(s_ + 1) * D], stg[:])

            # wk, wv: staged f32 behind wfc-odd on the scalar HWDGE queue.
            def hw_cast_w(src, nm, tag):
                wt = wpool.tile([128, NJ * D], BF16, tag=tag, name=nm)
                for s_ in range(NJ):
                    stg = scr_f.tile([128, D], F32, tag="sqscr",
                                     name=f"{nm}st{s_}")
                    nc.scalar.dma_start(stg[:], src.ap()[s_ * 128:(s_ + 1) * 128, :])
                    nc.vector.tensor_copy(wt[:, s_ * D:(s_ + 1) * D], stg[:])
                return wt

            wk_sb = hw_cast_w(wk, "wk_sb", "wko")
            wv_sb = hw_cast_w(wv, "wv_sb", "wvs")

            # persistent activations
            fusedT_b = [base.tile([128, NJ * CH], BF16, name=f"fusedT{b}")
                        for b in range(B)]
            fusedT_bv = [fT[:].rearrange("p (j t) -> p j t", j=NJ)
                         for fT in fusedT_b]
            qs_f32 = base.tile([QPC, D], BF16)
            kT = base.tile([128, NJ * CH], BF16, name="kT")
            kT_v = kT[:].rearrange("p (j t) -> p j t", j=NJ)
            v_b = base.tile([128, 4 * D], BF16, name="v_b")

            def rms_stats(src_ap):
                sq = scr_f.tile([128, D], F32, tag="sqscr")
                ssq = stat.tile([128, 1], F32, tag="ssq")
                nc.scalar.activation(sq[:], src_ap, AF.Square, accum_out=ssq[:])
                std = stat.tile([128, 1], F32, tag="std")
                nc.scalar.activation(std[:], ssq[:], AF.Sqrt, scale=1.0 / D,
                                     bias=eps_sb[:])
                rstd = stat.tile([128, 1], F32, tag="rstd")
                nc.vector.reciprocal(rstd[:], std[:])
                return rstd

            def transpose_to(dst_ap_3d, src_tile_ap, jlist):
                """PE-transpose 128x128 blocks into dst 3d view [128,len,128]."""
                ps = trps.tile([128, 512], BF16, tag="trp")
                for u, j in enumerate(jlist):
                    nc.tensor.transpose(
                        ps[:, u * 128:(u + 1) * 128],
                        src_tile_ap[:, j * 128:(j + 1) * 128],
                        id_sb[:],
                    )
                nc.vector.tensor_copy(
                    dst_ap_3d,
                    ps[:].rearrange("p (u x) -> p u x", u=len(jlist)),
                )

            # ---------------- phase 1: fc matmul ----------
            fps_tiles = {}

            def fc_group(tiles):
                """kk-outer over a group (used while wfc streams in)."""
                for key, _ in tiles:
                    fps_tiles[key] = mmps.tile([128, D], F32, tag="mm",
                                               name=f"fps{key}")
                for kk in range(NKK):
                    for key, st in tiles:
                        fps = fps_tiles[key]
                        for h in range(2):
                            nc.tensor.matmul(
                                fps[:, h * 512:(h + 1) * 512],
                                st[:, kk * 128:(kk + 1) * 128],
                                wfc_bf[:, kk * D + h * 512: kk * D + h * 512 + 512],
                                start=(kk == 0),
                                stop=(kk == NKK - 1),
                            )

            def fc_tile(key, st):
                """Tile-major full-rate fc for one 128-token tile."""
                fps = fps_tiles[key] = mmps.tile([128, D], F32, tag="mm",
                                                 name=f"fps{key}")
                for kk in range(NKK):
                    for h in range(2):
                        nc.tensor.matmul(
                            fps[:, h * 512:(h + 1) * 512],
                            st[:, kk * 128:(kk + 1) * 128],
                            wfc_bf[:, kk * D + h * 512: kk * D + h * 512 + 512],
                            start=(kk == 0),
                            stop=(kk == NKK - 1),
                        )

            def fused_epilogue(t):
                fps = fps_tiles[t]
                rstd = rms_stats(fps[:])
                fb = scr_bf.tile([128, D], BF16, tag="tmb")
                nc.vector.tensor_scalar(fb[:], fps[:], rstd[:], None, OP.mult)
                if apply_norm_weights:
                    nc.vector.tensor_tensor(fb[:], fb[:], whn_sb[:], op=OP.mult)
                bb, tl = divmod(t, 4)
                for g in range(2):
                    transpose_to(
                        fusedT_bv[bb][:, g * 4:(g + 1) * 4,
                                      tl * 128:(tl + 1) * 128],
                        fb[:],
                        [g * 4 + u for u in range(4)],
                    )

            def q_epilogue():
                fps = fps_tiles["t8"]
                rstd = rms_stats(fps[:])
                nc.vector.tensor_scalar(qs_f32[:], fps[:], rstd[:], None,
                                        OP.mult)
                if apply_norm_weights:
                    nc.vector.tensor_tensor(qs_f32[:], qs_f32[:], whn_sb[:],
                                            op=OP.mult)
                qhb = scr_bf.tile([128, D], BF16, tag="tmb")
                nc.vector.tensor_scalar(qhb[:], fps[:], rstd[:], None, OP.mult)
                if apply_norm_weights:
                    nc.vector.tensor_tensor(qhb[:], qhb[:], whn_sb[:],
                                            op=OP.mult)
                nc.vector.tensor_add(qhb[:], qhb[:], peq_bf[:])
                qht = scr_bf.tile([128, D], BF16, tag="tmb")
                qht_v = qht[:].rearrange("p (g x) -> p g x", g=2)
                for g in range(2):
                    transpose_to(
                        qht_v[:, g:g + 1, :].rearrange("p g x -> p (g x)")
                        .rearrange("p (u x) -> p u x", u=4),
                        qhb[:],
                        [g * 4 + u for u in range(4)],
                    )
                qps = mmps.tile([128, D], F32, tag="mm", name="qps")
                for j in range(NJ):
                    for h in range(2):
                        nc.tensor.matmul(
                            qps[:, h * 512:(h + 1) * 512],
                            qht[:, j * 128:(j + 1) * 128],
                            wq_sb[:, j * D + h * 512: j * D + h * 512 + 512],
                            start=(j == 0),
                            stop=(j == NJ - 1),
                        )
                qrstd = rms_stats(qps[:])
                qb = scr_bf.tile([128, D], BF16, tag="tmb")
                nc.vector.tensor_scalar(qb[:], qps[:], qrstd[:], None, OP.mult)
                if apply_norm_weights:
                    nc.vector.tensor_tensor(qb[:], qb[:], wqn_sb[:], op=OP.mult)
                nc.sync.dma_start(ag_in[:], qb[:])
                nc.gpsimd.collective_compute(
                    "AllGather", OP.bypass,
                    replica_groups=[list(range(NC))],
                    ins=[ag_in.opt()],
                    outs=[ag_out.opt()],
                )

            # -------- phase 2 helpers --------
            def k_tile(bb, tl):
                khb = scr_bf.tile([128, NJ * 128], BF16, tag="khb", bufs=1)
                nc.vector.tensor_add(
                    khb[:].rearrange("p (j x) -> p j x", j=NJ),
                    fusedT_bv[bb][:, :, tl * 128:(tl + 1) * 128],
                    pet_v[:, :, tl * 128:(tl + 1) * 128],
                )
                kps = mmps.tile([128, D], F32, tag="mm")
                for j in range(NJ):
                    for h in range(2):
                        nc.tensor.matmul(
                            kps[:, h * 512:(h + 1) * 512],
                            khb[:, j * 128:(j + 1) * 128],
                            wk_sb[:, j * D + h * 512: j * D + h * 512 + 512],
                            start=(j == 0),
                            stop=(j == NJ - 1),
                        )
                krstd = rms_stats(kps[:])
                kb = scr_bf.tile([128, D], BF16, tag="tmb")
                nc.vector.tensor_scalar(kb[:], kps[:], krstd[:], None, OP.mult)
                if apply_norm_weights:
                    nc.vector.tensor_tensor(kb[:], kb[:], wkn_sb[:], op=OP.mult)
                for g in range(2):
                    transpose_to(
                        kT_v[:, g * 4:(g + 1) * 4, tl * 128:(tl + 1) * 128],
                        kb[:],
                        [g * 4 + u for u in range(4)],
                    )

            def v_tile(bb, tl):
                vps = mmps.tile([128, D], F32, tag="mm")
                for j in range(NJ):
                    for h in range(2):
                        nc.tensor.matmul(
                            vps[:, h * 512:(h + 1) * 512],
                            fusedT_bv[bb][:, j:j + 1,
                                          tl * 128:(tl + 1) * 128]
                            .rearrange("p j x -> p (j x)"),
                            wv_sb[:, j * D + h * 512: j * D + h * 512 + 512],
                            start=(j == 0),
                            stop=(j == NJ - 1),
                        )
                nc.scalar.copy(v_b[:, tl * D:(tl + 1) * D], vps[:])

            def make_masks(lo):
                mk = scr_bf.tile([128, 4 * CH], BF16, tag="mask", bufs=1,
                                 name=f"mask{lo}")
                for u in range(4):
                    nc.vector.tensor_scalar(
                        mk[:, u * CH:(u + 1) * CH], iota_sb[:],
                        thr_sb[:, lo + u:lo + u + 1], MASK_NEG,
                        OP.is_gt, OP.mult)
                return mk

            def q_tile_T(i):
                bb, k2 = divmod(i, 4)
                qg = scr_bf.tile([128, D], BF16, tag="tmb")
                r0 = (2 * k2) * 128 + bb * 64
                r1 = (2 * k2 + 1) * 128 + bb * 64
                nc.sync.dma_start(qg[0:64, :], ag_out[r0:r0 + 64, :])
                nc.sync.dma_start(qg[64:128, :], ag_out[r1:r1 + 64, :])
                qTt = scr_bf.tile([128, NJ * 128], BF16, tag="qTt", bufs=2,
                                  name=f"qT{i}")
                qTt_v = qTt[:].rearrange("p (j x) -> p j x", j=NJ)
                for g in range(2):
                    transpose_to(
                        qTt_v[:, g * 4:(g + 1) * 4, :],
                        qg[:],
                        [g * 4 + u for u in range(4)],
                    )
                return qTt

            def attn_tile(i, qTt, mask_ap, rs_buf, row0):
                sps = mmps.tile([128, CH], F32, tag="mm")
                for j in range(NJ):
                    nc.tensor.matmul(
                        sps[:],
                        qTt[:, j * 128:(j + 1) * 128],
                        kT[:, j * CH:(j + 1) * CH],
                        start=(j == 0),
                        stop=(j == NJ - 1),
                    )
                sm = scr_f.tile([128, CH], F32, tag="mb", bufs=1)
                nc.vector.tensor_add(sm[:], sps[:], mask_ap)
                o_sb = scr_f.tile([128, D + 1], F16, tag="osb", bufs=2)
                lacc = stat.tile([128, 1], F32, tag="lacc")
                probs = scr_bf.tile([128, CH], BF16, tag="probs", bufs=2)
                nc.scalar.activation(probs[:], sm[:], AF.Exp, scale=SCALE,
                                     bias=ebias_sb[:], accum_out=lacc[:])
                nc.vector.tensor_copy(o_sb[:, D:D + 1], lacc[:])
                pT = scr_bf.tile([128, 512], BF16, tag="pT", bufs=1)
                transpose_to(pT[:].rearrange("p (u x) -> p u x", u=4),
                             probs[:], list(range(4)))
                ops_ = mmps.tile([128, D], F32, tag="mm")
                for u in range(4):
                    for h in range(2):
                        nc.tensor.matmul(
                            ops_[:, h * 512:(h + 1) * 512],
                            pT[:, u * 128:(u + 1) * 128],
                            v_b[:, u * D + h * 512: u * D + h * 512 + 512],
                            start=(u == 0),
                            stop=(u == 3),
                        )
                nc.vector.tensor_copy(o_sb[:, 0:D], ops_[:])
                return nc.sync.dma_start(rs_buf[row0:row0 + 128, :], o_sb[:])

            # ---------------- emission schedule ----------------
            # Group A streams with wfc; the rest is tile-major full-rate.
            fc_group([("t8", strip_q), (0, strips[0]), (1, strips[1])])
            q_epilogue()
            # strips t3..t7 queue behind the AllGather trigger
            for t in range(3, 8):
                strips[t] = load_strip(t, f"strip{t}")
            fused_epilogue(0)
            fused_epilogue(1)
            maskA = make_masks(0)
            fc_tile(2, strips[2])
            fused_epilogue(2)
            fc_tile(3, strips[3])
            fused_epilogue(3)
            for tl in range(4):
                k_tile(0, tl)
                v_tile(0, tl)
            fc_tile(4, strips[4])
            fused_epilogue(4)
            fc_tile(5, strips[5])
            fused_epilogue(5)
            fc_tile(6, strips[6])
            fused_epilogue(6)
            fc_tile(7, strips[7])
            fused_epilogue(7)

            for i in range(4):
                qTt = q_tile_T(i)
                attn_tile(i, qTt, maskA[:, (i % 4) * CH:(i % 4 + 1) * CH],
                          rs_inA, i * 128)
            nc.gpsimd.collective_compute(
                "ReduceScatter", OP.add,
                replica_groups=[list(range(NC))],
                ins=[rs_inA.opt()],
                outs=[rs_outA.opt()],
            )

            maskB = make_masks(4)
            for tl in range(4):
                k_tile(1, tl)
                v_tile(1, tl)
            for i in range(4, 8):
                qTt = q_tile_T(i)
                attn_tile(i, qTt, maskB[:, (i % 4) * CH:(i % 4 + 1) * CH],
                          rs_inB, (i - 4) * 128)
            nc.gpsimd.collective_compute(
                "ReduceScatter", OP.add,
                replica_groups=[list(range(NC))],
                ins=[rs_inB.opt()],
                outs=[rs_outB.opt()],
            )

            # wo load (SWDGE, reuses wk's slot) — after the RS_B trigger so
            # its WAR wait doesn't block the collectives on the gpsimd queue.
            wo_sb = wpool.tile([128, NJ * D], BF16, tag="wko", name="wo_sb")
            nc.gpsimd.dma_start(
                wo_sb[:].rearrange("p (s c) -> p s c", s=NJ), sliced(wo))

            # ---------------- epilogue for own 128 queries --------------
            fo = scr_f.tile([QPC, D + 1], F16, tag="fo", bufs=1)
            nc.sync.dma_start(fo[0:64, :], rs_outA[:])
            nc.sync.dma_start(fo[64:128, :], rs_outB[:])
            linv = stat.tile([128, 1], F32, tag="linv")
            nc.vector.reciprocal(linv[:], fo[:, D:D + 1])
            ao = scr_bf.tile([128, D], BF16, tag="tmb")
            nc.vector.tensor_scalar(ao[:], fo[:, 0:D], linv[:], None, OP.mult)
            aoT = scr_bf.tile([128, D], BF16, tag="tmb")
            aoT_v = aoT[:].rearrange("p (g x) -> p g x", g=2)
            for g in range(2):
                transpose_to(
                    aoT_v[:, g:g + 1, :].rearrange("p g x -> p (g x)")
                    .rearrange("p (u x) -> p u x", u=4),
                    ao[:],
                    [g * 4 + u for u in range(4)],
                )
            zps = mmps.tile([128, D], F32, tag="mm")
            for j in range(NJ):
                for h in range(2):
                    nc.tensor.matmul(
                        zps[:, h * 512:(h + 1) * 512],
                        aoT[:, j * 128:(j + 1) * 128],
                        wo_sb[:, j * D + h * 512: j * D + h * 512 + 512],
                        start=(j == 0),
                        stop=(j == NJ - 1),
                    )
            hh = scr_f.tile([128, D], F32, tag="sqscr")
            nc.vector.tensor_add(hh[:], qs_f32[:], zps[:])
            orstd = rms_stats(hh[:])
            yv = scr_f.tile([128, D], F32, tag="sqscr")
            nc.vector.tensor_scalar(yv[:], hh[:], orstd[:], None, OP.mult)
            if apply_norm_weights:
                nc.vector.tensor_tensor(yv[:], yv[:], won_sb[:], op=OP.mult)
            nc.sync.dma_start(out[:], yv[:])

    nc.compile()
    return nc


def _pe_table():
    half = D // 2
    inv_freq = np.exp(np.arange(half, dtype=np.float32)
                      * (-math.log(10000.0) / half))
    ang = np.arange(S, dtype=np.float32)[:, None] * inv_freq
    return np.concatenate([np.sin(ang), np.cos(ang)], axis=-1).astype(np.float32)


def _core_gidx(c):
    """Global query indices owned by core c (64 batch-0 then 64 batch-1)."""
    h = QPC // 2
    return np.concatenate([np.arange(c * h, (c + 1) * h),
                           N + np.arange(c * h, (c + 1) * h)])


def make_in_maps(np_inputs, apply_w=False):
    hid = np.asarray(np_inputs["hidden_states"], np.float32)
    pos = np.asarray(np_inputs["context_positions"])
    Wfc = np.ascontiguousarray(np.asarray(np_inputs["W_fc"], np.float32))
    Wq = np.ascontiguousarray(np.asarray(np_inputs["Wq"], np.float32))
    Wk = np.ascontiguousarray(np.asarray(np_inputs["Wk"], np.float32))
    Wv = np.ascontiguousarray(np.asarray(np_inputs["Wv"], np.float32))
    Wo = np.ascontiguousarray(np.asarray(np_inputs["Wo"], np.float32))

    x = hid.reshape(B, S, LD)
    p = np.clip(pos.astype(np.int64), 0, S - 1)
    p_flat = p.reshape(QT)
    PE = _pe_table()

    iota_np = np.tile(np.arange(CH, dtype=np.float32), (128, 1))
    ident_np = np.eye(128, dtype=np.float32).astype(ml_dtypes.bfloat16)

    in_maps = []
    for c in range(NC):
        sl = slice(c * CH, (c + 1) * CH)
        gidx = _core_gidx(c)
        # strip layout [128 part, kk, 128 tok]: elem (p,kk,c) = x^T[kk*128+p, c]
        x3 = np.empty((NT, 128, NKK, 128), np.float32)
        xq_cols = x[gidx // N, p_flat[gidx]].T          # [3072, 128]
        x3[0] = xq_cols.reshape(NKK, 128, 128).transpose(1, 0, 2)
        for t in range(8):
            bb, tl = divmod(t, 4)
            r0 = c * CH + tl * 128
            x3[1 + t] = (x[bb, r0:r0 + 128, :].T
                         .reshape(NKK, 128, 128).transpose(1, 0, 2))
        peq_a = np.ascontiguousarray(PE[p_flat[gidx]])
        pet_a = np.ascontiguousarray(PE[sl].T)
        thr_a = np.ascontiguousarray(
            (p_flat.astype(np.float32) - c * CH).reshape(NC, 128).T)
        m = {
            "x3": x3.reshape(NT * 128, NKK * 128),
            "wfc": Wfc, "wq": Wq, "wk": Wk, "wv": Wv, "wo": Wo,
            "pet": pet_a, "peq": peq_a, "thr": thr_a,
            "iota": iota_np, "ident": ident_np,
        }
        if apply_w:
            m["whn"] = np.tile(np.asarray(np_inputs["w_hidden_norm"], np.float32), (128, 1))
            m["wqn"] = np.tile(np.asarray(np_inputs["w_q_norm"], np.float32), (128, 1))
            m["wkn"] = np.tile(np.asarray(np_inputs["w_k_norm"], np.float32), (128, 1))
            m["won"] = np.tile(np.asarray(np_inputs["w_out_norm"], np.float32), (128, 1))
        in_maps.append(m)
    return in_maps


def assemble_out(results):
    y = np.zeros((QT, D), np.float32)
    for c in range(NC):
        y[_core_gidx(c)] = results[c]["out"]
    return y.reshape(B, N, D)


def kernel(**inputs) -> np.ndarray:
    w_h = np.asarray(inputs["w_hidden_norm"], np.float32)
    w_q = np.asarray(inputs["w_q_norm"], np.float32)
    w_k = np.asarray(inputs["w_k_norm"], np.float32)
    w_o = np.asarray(inputs["w_out_norm"], np.float32)
    apply_w = not (np.all(w_h == 1) and np.all(w_q == 1)
                   and np.all(w_k == 1) and np.all(w_o == 1))

    key = ("nc", apply_w)
    if key not in _CACHE:
        _CACHE[key] = _build(apply_w)
    nc = _CACHE[key]

    in_maps = make_in_maps(inputs, apply_w)

    trace = os.environ.get("KERNEL_TRACE", "0") == "1"
    if trace:
        try:
            import axon_prof
            axon_prof.install()
        except Exception:
            trace = False
    res = run_bass_kernel_spmd(nc, in_maps, list(range(NC)), trace=trace)
    global LAST_EXEC_NS
    LAST_EXEC_NS = res.exec_time_ns

    return assemble_out(res.results).astype(np.float32)


LAST_EXEC_NS = None


# revision 12
# speedup vs baseline: 1.3177x; 1.0182x over previous
"""Trainium2 Bass kernel for nn_CHSHistoryCrossAttentionFusion (8 NeuronCores, SPMD).

Decomposition (hardcoded for B=2, S=4096, L=3, D=1024, N=512, 8 cores):
  - History sequence-sharded: core c owns key positions [c*512, (c+1)*512) of
    each batch; it computes its chunk of fused/K/V from its x chunk.
  - Queries sharded 8-way for the Q path (64 batch-0 + 64 batch-1 queries per
    core); an AllGather replicates Q (bf16, small) so every core scores all
    1024 queries against its own K/V chunk.
  - Flash-style partial softmax per chunk WITHOUT max subtraction (Q/K are
    RMS-normalized so scores are bounded); causal mask applied additively
    before exp; exp carries a constant -ln(256) prescale so the (o,l)
    partials fit fp16.  Partials combine via two fp16 ReduceScatter-adds
    (one per batch), which also re-shard queries for the epilogue.
  - x is supplied host-side as 9 pre-transposed, partition-reblocked
    [128, 24*128] strips (8 history tiles + 1 gathered query tile) so the
    fc matmul needs no on-chip transposes and strip DMAs run 12KB-contiguous.
  - f32->bf16 conversion of x / wq / pet / peq / wo happens inside SWDGE
    cast-DMAs (gpsimd queue); wfc alternates f32 slices across the two
    HWDGE queues (+vector casts); wk/wv are staged behind wfc on the
    scalar queue.  The fc runs kk-outer over {query,t0,t1} while the wfc
    stream arrives (PE paced to DMA with sub-throttle-window gaps), then
    tile-major full-rate for t2..t7 (the in-order PE queue never waits on
    a not-yet-arrived strip).
  - All matmuls bf16 (fp32 accumulate); f32 in/out.
Host-side work is layout/indexing only.
"""

import math
import os

import numpy as np

try:
    import ml_dtypes
except ImportError:  # pragma: no cover
    ml_dtypes = None

import concourse.bacc as bacc
import concourse.mybir as mybir
import concourse.tile as tile
import concourse.tile_utils as tile_utils
from concourse.bass_utils import run_bass_kernel_spmd

# cayman has 208 KiB/partition usable; the default constant leaves 16 KiB idle
tile_utils.max_sbuf_usage = 208 * 1024

F32 = mybir.dt.float32
F16 = mybir.dt.float16
BF16 = mybir.dt.bfloat16
AF = mybir.ActivationFunctionType
OP = mybir.AluOpType

B, S, L, D = 2, 4096, 3, 1024
N = 512
NC = 8
CH = S // NC              # 512 keys per batch per core
LD = L * D                # 3072
QT = B * N                # 1024 global queries
QPC = QT // NC            # 128 queries per core (64 per batch)
NKK = LD // 128           # 24 contraction slices over 3072
NJ = D // 128             # 8 contraction slices over 1024
NT = 9                    # 8 history tiles + 1 query tile
RMS_EPS = 1e-6
SCALE = D ** -0.5
MASK_NEG = -1.0e6
EXP_BIAS = -math.log(256.0)

_CACHE = {}


def _build(apply_norm_weights: bool):
    nc = bacc.Bacc("TRN2", target_bir_lowering=False, num_devices=NC)

    # ---------------- I/O ----------------
    x3 = nc.dram_tensor("x3", [NT * 128, NKK * 128], F32, kind="ExternalInput")
    wfc = nc.dram_tensor("wfc", [LD, D], F32, kind="ExternalInput")
    wq = nc.dram_tensor("wq", [D, D], F32, kind="ExternalInput")
    wk = nc.dram_tensor("wk", [D, D], F32, kind="ExternalInput")
    wv = nc.dram_tensor("wv", [D, D], F32, kind="ExternalInput")
    wo = nc.dram_tensor("wo", [D, D], F32, kind="ExternalInput")
    pet = nc.dram_tensor("pet", [D, CH], F32, kind="ExternalInput")
    peq = nc.dram_tensor("peq", [QPC, D], F32, kind="ExternalInput")
    thr = nc.dram_tensor("thr", [128, NC], F32, kind="ExternalInput")
    iota = nc.dram_tensor("iota", [128, CH], F32, kind="ExternalInput")
    ident = nc.dram_tensor("ident", [128, 128], BF16, kind="ExternalInput")
    if apply_norm_weights:
        whn = nc.dram_tensor("whn", [128, D], F32, kind="ExternalInput")
        wqn = nc.dram_tensor("wqn", [128, D], F32, kind="ExternalInput")
        wkn = nc.dram_tensor("wkn", [128, D], F32, kind="ExternalInput")
        won = nc.dram_tensor("won", [128, D], F32, kind="ExternalInput")
    out = nc.dram_tensor("out", [QPC, D], F32, kind="ExternalOutput")

    def sliced(t):
        """DRAM [rows,cols] -> 3d AP [128, rows/128, cols] (partition-major)."""
        return t.ap().rearrange("(s p) c -> p s c", p=128)

    with tile.TileContext(nc) as tc:
        with (
            tc.tile_pool(name="dram", bufs=1, space="DRAM") as dram,
            tc.tile_pool(name="const", bufs=1) as constp,
            tc.tile_pool(name="stat", bufs=6) as stat,
            tc.tile_pool(name="wpool", bufs=1) as wpool,
            tc.tile_pool(name="strip", bufs=3) as stripp,
            tc.tile_pool(name="base", bufs=1) as base,
            tc.tile_pool(name="scr_bf", bufs=3) as scr_bf,
            tc.tile_pool(name="scr_f", bufs=3) as scr_f,
            tc.tile_pool(name="mmps", bufs=3, space="PSUM") as mmps,
            tc.tile_pool(name="trps", bufs=2, space="PSUM") as trps,
        ):
            # collective bounce buffers
            ag_in = dram.tile([QPC, D], BF16)
            ag_out = dram.tile([QT, D], BF16, addr_space="Shared")
            rs_inA = dram.tile([N, D + 1], F16)
            rs_outA = dram.tile([N // NC, D + 1], F16)
            rs_inB = dram.tile([N, D + 1], F16)
            rs_outB = dram.tile([N // NC, D + 1], F16)

            # small constants (sync queue, land in the first ~2us)
            id_sb = constp.tile([128, 128], BF16)
            nc.sync.dma_start(id_sb[:], ident[:])
            iota_sb = constp.tile([128, CH], F32)
            nc.sync.dma_start(iota_sb[:], iota[:])
            thr_sb = constp.tile([128, NC], F32)
            nc.sync.dma_start(thr_sb[:], thr[:])
            eps_sb = constp.tile([128, 1], F32)
            nc.vector.memset(eps_sb[:], RMS_EPS)
            ebias_sb = constp.tile([128, 1], F32)
            nc.vector.memset(ebias_sb[:], EXP_BIAS)
            if apply_norm_weights:
                whn_sb = constp.tile([128, D], F32)
                nc.sync.dma_start(whn_sb[:], whn[:])
                wqn_sb = constp.tile([128, D], F32)
                nc.sync.dma_start(wqn_sb[:], wqn[:])
                wkn_sb = constp.tile([128, D], F32)
                nc.sync.dma_start(wkn_sb[:], wkn[:])
                won_sb = constp.tile([128, D], F32)
                nc.sync.dma_start(won_sb[:], won[:])

            # ---------- bulk loads (part 1) ----------
            strips = [None] * 8

            def load_strip(t, name):
                st = stripp.tile([128, NKK * 128], BF16, tag="strip",
                                 name=name)
                idx = 0 if t is None else 1 + t
                nc.gpsimd.dma_start(
                    st[:], x3.ap()[idx * 128:(idx + 1) * 128, :])
                return st

            strip_q = load_strip(None, "strip_q")
            wq_sb = wpool.tile([128, NJ * D], BF16, tag="wqv", name="wq_sb")
            nc.gpsimd.dma_start(
                wq_sb[:].rearrange("p (s c) -> p s c", s=NJ), sliced(wq))
            peq_bf = wpool.tile([QPC, D], BF16)
            nc.gpsimd.dma_start(peq_bf[:], peq.ap())
            for t in range(4):
                strips[t] = load_strip(t, f"strip{t}")
            pet_bf = wpool.tile([128, NJ * CH], BF16)     # [d_lo, j*512+tc]
            nc.gpsimd.dma_start(
                pet_bf[:].rearrange("p (s c) -> p s c", s=NJ), sliced(pet))
            pet_v = pet_bf[:].rearrange("p (j t) -> p j t", j=NJ)

            # wfc: f32 slices alternating across the two HWDGE queues,
            # cast on vector.  Stage tiles share the scr_f "sqscr" tag.
            wfc_bf = wpool.tile([128, NKK * D], BF16)
            for s_ in range(NKK):
                stg = scr_f.tile([128, D], F32, tag="sqscr", name=f"wfcst{s_}")
                eng = nc.sync if s_ % 2 == 0 else nc.scalar
                eng.dma_start(stg[:], wfc.ap()[s_ * 128:(s_ + 1) * 128, :])
                nc.vector.tensor_copy(wfc_bf[:, s_ * D:(s_ + 1) * D], stg[:])

            # wk, wv: staged f32 behind wfc-odd on the scalar HWDGE queue.
            def hw_cast_w(src, nm, tag):
                wt = wpool.tile([128, NJ * D], BF16, tag=tag, name=nm)
                for s_ in range(NJ):
                    stg = scr_f.tile([128, D], F32, tag="sqscr",
                                     name=f"{nm}st{s_}")
                    nc.scalar.dma_start(stg[:], src.ap()[s_ * 128:(s_ + 1) * 128, :])
                    nc.vector.tensor_copy(wt[:, s_ * D:(s_ + 1) * D], stg[:])
                return wt

            wk_sb = hw_cast_w(wk, "wk_sb", "wko")
            wv_sb = hw_cast_w(wv, "wv_sb", "wvs")

            # persistent activations
            fusedT_b = [base.tile([128, NJ * CH], BF16, name=f"fusedT{b}")
                        for b in range(B)]
            fusedT_bv = [fT[:].rearrange("p (j t) -> p j t", j=NJ)
                         for fT in fusedT_b]
            qs_f32 = base.tile([QPC, D], F32)
            kT = base.tile([128, NJ * CH], BF16, name="kT")
            kT_v = kT[:].rearrange("p (j t) -> p j t", j=NJ)
            v_b = base.tile([128, 4 * D], BF16, name="v_b")

            def rms_stats(src_ap):
                sq = scr_f.tile([128, D], F32, tag="sqscr")
                ssq = stat.tile([128, 1], F32, tag="ssq")
                nc.scalar.activation(sq[:], src_ap, AF.Square, accum_out=ssq[:])
                std = stat.tile([128, 1], F32, tag="std")
                nc.scalar.activation(std[:], ssq[:], AF.Sqrt, scale=1.0 / D,
                                     bias=eps_sb[:])
                rstd = stat.tile([128, 1], F32, tag="rstd")
                nc.vector.reciprocal(rstd[:], std[:])
                return rstd

            def transpose_to(dst_ap_3d, src_tile_ap, jlist):
                """PE-transpose 128x128 blocks into dst 3d view [128,len,128]."""
                ps = trps.tile([128, 512], BF16, tag="trp")
                for u, j in enumerate(jlist):
                    nc.tensor.transpose(
                        ps[:, u * 128:(u + 1) * 128],
                        src_tile_ap[:, j * 128:(j + 1) * 128],
                        id_sb[:],
                    )
                nc.vector.tensor_copy(
                    dst_ap_3d,
                    ps[:].rearrange("p (u x) -> p u x", u=len(jlist)),
                )

            # ---------------- phase 1: fc matmul ----------
            fps_tiles = {}

            def fc_group(tiles):
                """kk-outer over a group (used while wfc streams in)."""
                for key, _ in tiles:
                    fps_tiles[key] = mmps.tile([128, D], F32, tag="mm",
                                               name=f"fps{key}")
                for kk in range(NKK):
                    for key, st in tiles:
                        fps = fps_tiles[key]
                        for h in range(2):
                            nc.tensor.matmul(
                                fps[:, h * 512:(h + 1) * 512],
                                st[:, kk * 128:(kk + 1) * 128],
                                wfc_bf[:, kk * D + h * 512: kk * D + h * 512 + 512],
                                start=(kk == 0),
                                stop=(kk == NKK - 1),
                            )

            def fc_tile(key, st):
                """Tile-major full-rate fc for one 128-token tile."""
                fps = fps_tiles[key] = mmps.tile([128, D], F32, tag="mm",
                                                 name=f"fps{key}")
                for kk in range(NKK):
                    for h in range(2):
                        nc.tensor.matmul(
                            fps[:, h * 512:(h + 1) * 512],
                            st[:, kk * 128:(kk + 1) * 128],
                            wfc_bf[:, kk * D + h * 512: kk * D + h * 512 + 512],
                            start=(kk == 0),
                            stop=(kk == NKK - 1),
                        )

            def fused_epilogue(t):
                fps = fps_tiles[t]
                rstd = rms_stats(fps[:])
                fb = scr_bf.tile([128, D], BF16, tag="tmb")
                nc.vector.tensor_scalar(fb[:], fps[:], rstd[:], None, OP.mult)
                if apply_norm_weights:
                    nc.vector.tensor_tensor(fb[:], fb[:], whn_sb[:], op=OP.mult)
                bb, tl = divmod(t, 4)
                for g in range(2):
                    transpose_to(
                        fusedT_bv[bb][:, g * 4:(g + 1) * 4,
                                      tl * 128:(tl + 1) * 128],
                        fb[:],
                        [g * 4 + u for u in range(4)],
                    )

            def q_epilogue():
                fps = fps_tiles["t8"]
                rstd = rms_stats(fps[:])
                nc.vector.tensor_scalar(qs_f32[:], fps[:], rstd[:], None,
                                        OP.mult)
                if apply_norm_weights:
                    nc.vector.tensor_tensor(qs_f32[:], qs_f32[:], whn_sb[:],
                                            op=OP.mult)
                qhb = scr_bf.tile([128, D], BF16, tag="tmb")
                nc.vector.tensor_scalar(qhb[:], fps[:], rstd[:], None, OP.mult)
                if apply_norm_weights:
                    nc.vector.tensor_tensor(qhb[:], qhb[:], whn_sb[:],
                                            op=OP.mult)
                nc.vector.tensor_add(qhb[:], qhb[:], peq_bf[:])
                qht = scr_bf.tile([128, D], BF16, tag="tmb")
                qht_v = qht[:].rearrange("p (g x) -> p g x", g=2)
                for g in range(2):
                    transpose_to(
                        qht_v[:, g:g + 1, :].rearrange("p g x -> p (g x)")
                        .rearrange("p (u x) -> p u x", u=4),
                        qhb[:],
                        [g * 4 + u for u in range(4)],
                    )
                qps = mmps.tile([128, D], F32, tag="mm", name="qps")
                for j in range(NJ):
                    for h in range(2):
                        nc.tensor.matmul(
                            qps[:, h * 512:(h + 1) * 512],
                            qht[:, j * 128:(j + 1) * 128],
                            wq_sb[:, j * D + h * 512: j * D + h * 512 + 512],
                            start=(j == 0),
                            stop=(j == NJ - 1),
                        )
                qrstd = rms_stats(qps[:])
                qb = scr_bf.tile([128, D], BF16, tag="tmb")
                nc.vector.tensor_scalar(qb[:], qps[:], qrstd[:], None, OP.mult)
                if apply_norm_weights:
                    nc.vector.tensor_tensor(qb[:], qb[:], wqn_sb[:], op=OP.mult)
                nc.sync.dma_start(ag_in[:], qb[:])
                nc.gpsimd.collective_compute(
                    "AllGather", OP.bypass,
                    replica_groups=[list(range(NC))],
                    ins=[ag_in.opt()],
                    outs=[ag_out.opt()],
                )

            # -------- phase 2 helpers --------
            def k_tile(bb, tl):
                khb = scr_bf.tile([128, NJ * 128], BF16, tag="khb", bufs=2)
                nc.vector.tensor_add(
                    khb[:].rearrange("p (j x) -> p j x", j=NJ),
                    fusedT_bv[bb][:, :, tl * 128:(tl + 1) * 128],
                    pet_v[:, :, tl * 128:(tl + 1) * 128],
                )
                kps = mmps.tile([128, D], F32, tag="mm")
                for j in range(NJ):
                    for h in range(2):
                        nc.tensor.matmul(
                            kps[:, h * 512:(h + 1) * 512],
                            khb[:, j * 128:(j + 1) * 128],
                            wk_sb[:, j * D + h * 512: j * D + h * 512 + 512],
                            start=(j == 0),
                            stop=(j == NJ - 1),
                        )
                krstd = rms_stats(kps[:])
                kb = scr_bf.tile([128, D], BF16, tag="tmb")
                nc.vector.tensor_scalar(kb[:], kps[:], krstd[:], None, OP.mult)
                if apply_norm_weights:
                    nc.vector.tensor_tensor(kb[:], kb[:], wkn_sb[:], op=OP.mult)
                for g in range(2):
                    transpose_to(
                        kT_v[:, g * 4:(g + 1) * 4, tl * 128:(tl + 1) * 128],
                        kb[:],
                        [g * 4 + u for u in range(4)],
                    )

            def v_tile(bb, tl):
                vps = mmps.tile([128, D], F32, tag="mm")
                for j in range(NJ):
                    for h in range(2):
                        nc.tensor.matmul(
                            vps[:, h * 512:(h + 1) * 512],
                            fusedT_bv[bb][:, j:j + 1,
                                          tl * 128:(tl + 1) * 128]
                            .rearrange("p j x -> p (j x)"),
                            wv_sb[:, j * D + h * 512: j * D + h * 512 + 512],
                            start=(j == 0),
                            stop=(j == NJ - 1),
                        )
                nc.vector.tensor_copy(v_b[:, tl * D:(tl + 1) * D], vps[:])

            def q_tile_T(i):
                bb, k2 = divmod(i, 4)
                qg = scr_bf.tile([128, D], BF16, tag="tmb")
                r0 = (2 * k2) * 128 + bb * 64
                r1 = (2 * k2 + 1) * 128 + bb * 64
                nc.sync.dma_start(qg[0:64, :], ag_out[r0:r0 + 64, :])
                nc.sync.dma_start(qg[64:128, :], ag_out[r1:r1 + 64, :])
                qTt = scr_bf.tile([128, NJ * 128], BF16, tag="qTt", bufs=3,
                                  name=f"qT{i}")
                qTt_v = qTt[:].rearrange("p (j x) -> p j x", j=NJ)
                for g in range(2):
                    transpose_to(
                        qTt_v[:, g * 4:(g + 1) * 4, :],
                        qg[:],
                        [g * 4 + u for u in range(4)],
                    )
                return qTt

            def attn_tile(i, qTt, mask_ap, rs_buf, row0):
                sps = mmps.tile([128, CH], F32, tag="mm")
                for j in range(NJ):
                    nc.tensor.matmul(
                        sps[:],
                        qTt[:, j * 128:(j + 1) * 128],
                        kT[:, j * CH:(j + 1) * CH],
                        start=(j == 0),
                        stop=(j == NJ - 1),
                    )
                mb = scr_f.tile([128, CH], F32, tag="mb", bufs=2)
                nc.vector.tensor_scalar(mb[:], iota_sb[:],
                                        thr_sb[:, i:i + 1], MASK_NEG,
                                        OP.is_gt, OP.mult)
                sm = scr_f.tile([128, CH], F32, tag="mb", bufs=2)
                nc.vector.tensor_add(sm[:], sps[:], mb[:])
                o_sb = scr_f.tile([128, D + 1], F16, tag="osb", bufs=3)
                lacc = stat.tile([128, 1], F32, tag="lacc")
                probs = scr_bf.tile([128, CH], BF16, tag="probs", bufs=2)
                nc.scalar.activation(probs[:], sm[:], AF.Exp, scale=SCALE,
                                     bias=ebias_sb[:], accum_out=lacc[:])
                nc.vector.tensor_copy(o_sb[:, D:D + 1], lacc[:])
                pT = scr_bf.tile([128, 512], BF16, tag="pT", bufs=2)
                transpose_to(pT[:].rearrange("p (u x) -> p u x", u=4),
                             probs[:], list(range(4)))
                ops_ = mmps.tile([128, D], F32, tag="mm")
                for u in range(4):
                    for h in range(2):
                        nc.tensor.matmul(
                            ops_[:, h * 512:(h + 1) * 512],
                            pT[:, u * 128:(u + 1) * 128],
                            v_b[:, u * D + h * 512: u * D + h * 512 + 512],
                            start=(u == 0),
                            stop=(u == 3),
                        )
                nc.vector.tensor_copy(o_sb[:, 0:D], ops_[:])
                return nc.scalar.dma_start(rs_buf[row0:row0 + 128, :], o_sb[:])

            # ---------------- emission schedule ----------------
            # Group A streams with wfc; the rest is tile-major full-rate.
            fc_group([("t8", strip_q), (0, strips[0]), (1, strips[1])])
            q_epilogue()
            # strips t4..t7 queue behind the AllGather trigger
            for t in range(4, 8):
                strips[t] = load_strip(t, f"strip{t}")
            fused_epilogue(0)
            fused_epilogue(1)
            fc_group([(2, strips[2]), (3, strips[3])])
            fused_epilogue(2)
            fused_epilogue(3)

            for tl in range(4):
                k_tile(0, tl)
                v_tile(0, tl)

            for i in range(4):
                qTt = q_tile_T(i)
                attn_tile(i, qTt, None, rs_inA, i * 128)
            nc.gpsimd.collective_compute(
                "ReduceScatter", OP.add,
                replica_groups=[list(range(NC))],
                ins=[rs_inA.opt()],
                outs=[rs_outA.opt()],
            )

            fc_group([(4, strips[4]), (5, strips[5]), (6, strips[6])])
            fused_epilogue(4)
            fused_epilogue(5)
            fused_epilogue(6)
            fc_group([(7, strips[7])])
            fused_epilogue(7)
            for tl in range(4):
                k_tile(1, tl)
                v_tile(1, tl)
            for i in range(4, 8):
                qTt = q_tile_T(i)
                attn_tile(i, qTt, None, rs_inB, (i - 4) * 128)
            nc.gpsimd.collective_compute(
                "ReduceScatter", OP.add,
                replica_groups=[list(range(NC))],
                ins=[rs_inB.opt()],
                outs=[rs_outB.opt()],
            )

            # wo load (SWDGE, reuses wk's slot) — after the RS_B trigger so
            # its WAR wait doesn't block the collectives on the gpsimd queue.
            wo_sb = wpool.tile([128, NJ * D], BF16, tag="wko", name="wo_sb")
            nc.gpsimd.dma_start(
                wo_sb[:].rearrange("p (s c) -> p s c", s=NJ), sliced(wo))

            # ---------------- epilogue for own 128 queries --------------
            fo = scr_f.tile([QPC, D + 1], F16, tag="fo", bufs=1)
            nc.sync.dma_start(fo[0:64, :], rs_outA[:])
            nc.sync.dma_start(fo[64:128, :], rs_outB[:])
            linv = stat.tile([128, 1], F32, tag="linv")
            nc.vector.reciprocal(linv[:], fo[:, D:D + 1])
            ao = scr_bf.tile([128, D], BF16, tag="tmb")
            nc.vector.tensor_scalar(ao[:], fo[:, 0:D], linv[:], None, OP.mult)
            aoT = scr_bf.tile([128, D], BF16, tag="tmb")
            aoT_v = aoT[:].rearrange("p (g x) -> p g x", g=2)
            for g in range(2):
                transpose_to(
                    aoT_v[:, g:g + 1, :].rearrange("p g x -> p (g x)")
                    .rearrange("p (u x) -> p u x", u=4),
                    ao[:],
                    [g * 4 + u for u in range(4)],
                )
            zps = mmps.tile([128, D], F32, tag="mm")
            for j in range(NJ):
                for h in range(2):
                    nc.tensor.matmul(
                        zps[:, h * 512:(h + 1) * 512],
                        aoT[:, j * 128:(j + 1) * 128],
                        wo_sb[:, j * D + h * 512: j * D + h * 512 + 512],
                        start=(j == 0),
                        stop=(j == NJ - 1),
                    )
            hh = scr_f.tile([128, D], F32, tag="sqscr")
            nc.vector.tensor_add(hh[:], qs_f32[:], zps[:])
            orstd = rms_stats(hh[:])
            yv = scr_f.tile([128, D], F32, tag="sqscr")
            nc.vector.tensor_scalar(yv[:], hh[:], orstd[:], None, OP.mult)
            if apply_norm_weights:
                nc.vector.tensor_tensor(yv[:], yv[:], won_sb[:], op=OP.mult)
            nc.sync.dma_start(out[:], yv[:])

    nc.compile()
    return nc


def _pe_table():
    half = D // 2
    inv_freq = np.exp(np.arange(half, dtype=np.float32)
                      * (-math.log(10000.0) / half))
    ang = np.arange(S, dtype=np.float32)[:, None] * inv_freq
    return np.concatenate([np.sin(ang), np.cos(ang)], axis=-1).astype(np.float32)


def _core_gidx(c):
    """Global query indices owned by core c (64 batch-0 then 64 batch-1)."""
    h = QPC // 2
    return np.concatenate([np.arange(c * h, (c + 1) * h),
                           N + np.arange(c * h, (c + 1) * h)])


def make_in_maps(np_inputs, apply_w=False):
    hid = np.asarray(np_inputs["hidden_states"], np.float32)
    pos = np.asarray(np_inputs["context_positions"])
    Wfc = np.ascontiguousarray(np.asarray(np_inputs["W_fc"], np.float32))
    Wq = np.ascontiguousarray(np.asarray(np_inputs["Wq"], np.float32))
    Wk = np.ascontiguousarray(np.asarray(np_inputs["Wk"], np.float32))
    Wv = np.ascontiguousarray(np.asarray(np_inputs["Wv"], np.float32))
    Wo = np.ascontiguousarray(np.asarray(np_inputs["Wo"], np.float32))

    x = hid.reshape(B, S, LD)
    p = np.clip(pos.astype(np.int64), 0, S - 1)
    p_flat = p.reshape(QT)
    PE = _pe_table()

    iota_np = np.tile(np.arange(CH, dtype=np.float32), (128, 1))
    ident_np = np.eye(128, dtype=np.float32).astype(ml_dtypes.bfloat16)

    in_maps = []
    for c in range(NC):
        sl = slice(c * CH, (c + 1) * CH)
        gidx = _core_gidx(c)
        # strip layout [128 part, kk, 128 tok]: elem (p,kk,c) = x^T[kk*128+p, c]
        x3 = np.empty((NT, 128, NKK, 128), np.float32)
        xq_cols = x[gidx // N, p_flat[gidx]].T          # [3072, 128]
        x3[0] = xq_cols.reshape(NKK, 128, 128).transpose(1, 0, 2)
        for t in range(8):
            bb, tl = divmod(t, 4)
            r0 = c * CH + tl * 128
            x3[1 + t] = (x[bb, r0:r0 + 128, :].T
                         .reshape(NKK, 128, 128).transpose(1, 0, 2))
        peq_a = np.ascontiguousarray(PE[p_flat[gidx]])
        pet_a = np.ascontiguousarray(PE[sl].T)
        thr_a = np.ascontiguousarray(
            (p_flat.astype(np.float32) - c * CH).reshape(NC, 128).T)
        m = {
            "x3": x3.reshape(NT * 128, NKK * 128),
            "wfc": Wfc, "wq": Wq, "wk": Wk, "wv": Wv, "wo": Wo,
            "pet": pet_a, "peq": peq_a, "thr": thr_a,
            "iota": iota_np, "ident": ident_np,
        }
        if apply_w:
            m["whn"] = np.tile(np.asarray(np_inputs["w_hidden_norm"], np.float32), (128, 1))
            m["wqn"] = np.tile(np.asarray(np_inputs["w_q_norm"], np.float32), (128, 1))
            m["wkn"] = np.tile(np.asarray(np_inputs["w_k_norm"], np.float32), (128, 1))
            m["won"] = np.tile(np.asarray(np_inputs["w_out_norm"], np.float32), (128, 1))
        in_maps.append(m)
    return in_maps


def assemble_out(results):
    y = np.zeros((QT, D), np.float32)
    for c in range(NC):
        y[_core_gidx(c)] = results[c]["out"]
    return y.reshape(B, N, D)


def kernel(**inputs) -> np.ndarray:
    w_h = np.asarray(inputs["w_hidden_norm"], np.float32)
    w_q = np.asarray(inputs["w_q_norm"], np.float32)
    w_k = np.asarray(inputs["w_k_norm"], np.float32)
    w_o = np.asarray(inputs["w_out_norm"], np.float32)
    apply_w = not (np.all(w_h == 1) and np.all(w_q == 1)
                   and np.all(w_k == 1) and np.all(w_o == 1))

    key = ("nc", apply_w)
    if key not in _CACHE:
        _CACHE[key] = _build(apply_w)
    nc = _CACHE[key]

    in_maps = make_in_maps(inputs, apply_w)

    trace = os.environ.get("KERNEL_TRACE", "0") == "1"
    if trace:
        try:
            import axon_prof
            axon_prof.install()
        except Exception:
            trace = False
    res = run_bass_kernel_spmd(nc, in_maps, list(range(NC)), trace=trace)
    global LAST_EXEC_NS
    LAST_EXEC_NS = res.exec_time_ns

    return assemble_out(res.results).astype(np.float32)


LAST_EXEC_NS = None


# revision 13
# speedup vs baseline: 1.4290x; 1.0844x over previous
"""Trainium2 Bass kernel for nn_CHSHistoryCrossAttentionFusion (8 NeuronCores, SPMD).

Decomposition (hardcoded for B=2, S=4096, L=3, D=1024, N=512, 8 cores):
  - History sequence-sharded: core c owns key positions [c*512, (c+1)*512) of
    each batch; it computes its chunk of fused/K/V from its x chunk.
  - Queries sharded 8-way for the Q path (64 batch-0 + 64 batch-1 queries per
    core); an AllGather replicates Q (bf16, small) so every core scores all
    1024 queries against its own K/V chunk.
  - Flash-style partial softmax per chunk WITHOUT max subtraction (Q/K are
    RMS-normalized so scores are bounded); causal mask applied additively
    before exp; exp carries a constant -ln(256) prescale so the (o,l)
    partials fit fp16.  Partials combine via two fp16 ReduceScatter-adds
    (one per batch), which also re-shard queries for the epilogue.
  - x is supplied host-side as 9 pre-transposed, partition-reblocked
    [128, 24*128] strips (8 history tiles + 1 gathered query tile) so the
    fc matmul needs no on-chip transposes and strip DMAs run 12KB-contiguous.
  - f32->bf16 conversion of x / wq / pet / peq / wo happens inside SWDGE
    cast-DMAs (gpsimd queue); wfc alternates f32 slices across the two
    HWDGE queues (+vector casts); wk/wv are staged behind wfc on the
    scalar queue.  The fc runs kk-outer over {query,t0,t1} while the wfc
    stream arrives (PE paced to DMA with sub-throttle-window gaps), then
    tile-major full-rate for t2..t7 (the in-order PE queue never waits on
    a not-yet-arrived strip).
  - All matmuls bf16 (fp32 accumulate); f32 in/out.
Host-side work is layout/indexing only.
"""

import math
import os

import numpy as np

try:
    import ml_dtypes
except ImportError:  # pragma: no cover
    ml_dtypes = None

import concourse.bacc as bacc
import concourse.mybir as mybir
import concourse.tile as tile
import concourse.tile_utils as tile_utils
from concourse.bass_utils import run_bass_kernel_spmd

# cayman has 208 KiB/partition usable; the default constant leaves 16 KiB idle
tile_utils.max_sbuf_usage = 208 * 1024

F32 = mybir.dt.float32
F16 = mybir.dt.float16
BF16 = mybir.dt.bfloat16
AF = mybir.ActivationFunctionType
OP = mybir.AluOpType

B, S, L, D = 2, 4096, 3, 1024
N = 512
NC = 8
CH = S // NC              # 512 keys per batch per core
LD = L * D                # 3072
QT = B * N                # 1024 global queries
QPC = QT // NC            # 128 queries per core (64 per batch)
NKK = LD // 128           # 24 contraction slices over 3072
NJ = D // 128             # 8 contraction slices over 1024
NT = 9                    # 8 history tiles + 1 query tile
RMS_EPS = 1e-6
SCALE = D ** -0.5
MASK_NEG = -1.0e6
EXP_BIAS = -math.log(256.0)

_CACHE = {}


def _build(apply_norm_weights: bool):
    nc = bacc.Bacc("TRN2", target_bir_lowering=False, num_devices=NC)

    # ---------------- I/O ----------------
    x3 = nc.dram_tensor("x3", [NT * 128, NKK * 128], F32, kind="ExternalInput")
    wfc = nc.dram_tensor("wfc", [LD, D], F32, kind="ExternalInput")
    wq = nc.dram_tensor("wq", [D, D], F32, kind="ExternalInput")
    wk = nc.dram_tensor("wk", [D, D], F32, kind="ExternalInput")
    wv = nc.dram_tensor("wv", [D, D], F32, kind="ExternalInput")
    wo = nc.dram_tensor("wo", [D, D], F32, kind="ExternalInput")
    pet = nc.dram_tensor("pet", [D, CH], F32, kind="ExternalInput")
    peq = nc.dram_tensor("peq", [QPC, D], F32, kind="ExternalInput")
    thr = nc.dram_tensor("thr", [128, NC], F32, kind="ExternalInput")
    iota = nc.dram_tensor("iota", [128, CH], F32, kind="ExternalInput")
    ident = nc.dram_tensor("ident", [128, 128], BF16, kind="ExternalInput")
    if apply_norm_weights:
        whn = nc.dram_tensor("whn", [128, D], F32, kind="ExternalInput")
        wqn = nc.dram_tensor("wqn", [128, D], F32, kind="ExternalInput")
        wkn = nc.dram_tensor("wkn", [128, D], F32, kind="ExternalInput")
        won = nc.dram_tensor("won", [128, D], F32, kind="ExternalInput")
    out = nc.dram_tensor("out", [QPC, D], F32, kind="ExternalOutput")

    def sliced(t):
        """DRAM [rows,cols] -> 3d AP [128, rows/128, cols] (partition-major)."""
        return t.ap().rearrange("(s p) c -> p s c", p=128)

    with tile.TileContext(nc) as tc:
        with (
            tc.tile_pool(name="dram", bufs=1, space="DRAM") as dram,
            tc.tile_pool(name="const", bufs=1) as constp,
            tc.tile_pool(name="stat", bufs=6) as stat,
            tc.tile_pool(name="wpool", bufs=1) as wpool,
            tc.tile_pool(name="strip", bufs=3) as stripp,
            tc.tile_pool(name="base", bufs=1) as base,
            tc.tile_pool(name="scr_bf", bufs=3) as scr_bf,
            tc.tile_pool(name="scr_f", bufs=3) as scr_f,
            tc.tile_pool(name="mmps", bufs=3, space="PSUM") as mmps,
            tc.tile_pool(name="trps", bufs=2, space="PSUM") as trps,
        ):
            # collective bounce buffers
            ag_in = dram.tile([QPC, D], BF16)
            ag_out = dram.tile([QT, D], BF16, addr_space="Shared")
            rs_inA = dram.tile([N, D + 1], F16)
            rs_outA = dram.tile([N // NC, D + 1], F16)
            rs_inB = dram.tile([N, D + 1], F16)
            rs_outB = dram.tile([N // NC, D + 1], F16)

            # small constants (sync queue, land in the first ~2us)
            id_sb = constp.tile([128, 128], BF16)
            nc.sync.dma_start(id_sb[:], ident[:])
            iota_sb = constp.tile([128, CH], F32)
            nc.sync.dma_start(iota_sb[:], iota[:])
            thr_sb = constp.tile([128, NC], F32)
            nc.sync.dma_start(thr_sb[:], thr[:])
            eps_sb = constp.tile([128, 1], F32)
            nc.vector.memset(eps_sb[:], RMS_EPS)
            ebias_sb = constp.tile([128, 1], F32)
            nc.vector.memset(ebias_sb[:], EXP_BIAS)
            if apply_norm_weights:
                whn_sb = constp.tile([128, D], F32)
                nc.sync.dma_start(whn_sb[:], whn[:])
                wqn_sb = constp.tile([128, D], F32)
                nc.sync.dma_start(wqn_sb[:], wqn[:])
                wkn_sb = constp.tile([128, D], F32)
                nc.sync.dma_start(wkn_sb[:], wkn[:])
                won_sb = constp.tile([128, D], F32)
                nc.sync.dma_start(won_sb[:], won[:])

            # ---------- bulk loads (part 1) ----------
            strips = [None] * 8

            def load_strip(t, name):
                st = stripp.tile([128, NKK * 128], BF16, tag="strip",
                                 name=name)
                idx = 0 if t is None else 1 + t
                nc.gpsimd.dma_start(
                    st[:], x3.ap()[idx * 128:(idx + 1) * 128, :])
                return st

            wfc_bf = wpool.tile([128, NKK * D], BF16)

            def load_wfc_chunk(c):
                # 2 contraction slices (1MB f32) per SWDGE cast-DMA
                nc.gpsimd.dma_start(
                    wfc_bf[:, 2 * c * D:(2 * c + 2) * D]
                    .rearrange("p (s c) -> p s c", s=2),
                    sliced(wfc)[:, 2 * c:2 * c + 2, :])

            load_wfc_chunk(0)
            load_wfc_chunk(1)
            strip_q = load_strip(None, "strip_q")
            load_wfc_chunk(2)
            strips[0] = load_strip(0, "strip0")
            load_wfc_chunk(3)
            strips[1] = load_strip(1, "strip1")
            load_wfc_chunk(4)
            load_wfc_chunk(5)
            wq_sb = wpool.tile([128, NJ * D], BF16, tag="wqv", name="wq_sb")
            nc.gpsimd.dma_start(
                wq_sb[:].rearrange("p (s c) -> p s c", s=NJ), sliced(wq))
            load_wfc_chunk(6)
            load_wfc_chunk(7)
            peq_bf = wpool.tile([QPC, D], BF16)
            nc.gpsimd.dma_start(peq_bf[:], peq.ap())
            load_wfc_chunk(8)
            load_wfc_chunk(9)
            strips[2] = load_strip(2, "strip2")
            load_wfc_chunk(10)
            load_wfc_chunk(11)
            strips[3] = load_strip(3, "strip3")
            pet_bf = wpool.tile([128, NJ * CH], BF16)     # [d_lo, j*512+tc]
            nc.gpsimd.dma_start(
                pet_bf[:].rearrange("p (s c) -> p s c", s=NJ), sliced(pet))
            pet_v = pet_bf[:].rearrange("p (j t) -> p j t", j=NJ)

            # wk, wv: staged f32 behind wfc-odd on the scalar HWDGE queue.
            def hw_cast_w(src, nm, tag):
                wt = wpool.tile([128, NJ * D], BF16, tag=tag, name=nm)
                for s_ in range(NJ):
                    stg = scr_f.tile([128, D], F32, tag="sqscr",
                                     name=f"{nm}st{s_}")
                    nc.scalar.dma_start(stg[:], src.ap()[s_ * 128:(s_ + 1) * 128, :])
                    nc.vector.tensor_copy(wt[:, s_ * D:(s_ + 1) * D], stg[:])
                return wt

            wk_sb = hw_cast_w(wk, "wk_sb", "wko")
            wv_sb = hw_cast_w(wv, "wv_sb", "wvs")

            # persistent activations
            fusedT_b = [base.tile([128, NJ * CH], BF16, name=f"fusedT{b}")
                        for b in range(B)]
            fusedT_bv = [fT[:].rearrange("p (j t) -> p j t", j=NJ)
                         for fT in fusedT_b]
            qs_f32 = base.tile([QPC, D], F32)
            kT = base.tile([128, NJ * CH], BF16, name="kT")
            kT_v = kT[:].rearrange("p (j t) -> p j t", j=NJ)
            v_b = base.tile([128, 4 * D], BF16, name="v_b")

            def rms_stats(src_ap):
                sq = scr_f.tile([128, D], F32, tag="sqscr")
                ssq = stat.tile([128, 1], F32, tag="ssq")
                nc.scalar.activation(sq[:], src_ap, AF.Square, accum_out=ssq[:])
                std = stat.tile([128, 1], F32, tag="std")
                nc.scalar.activation(std[:], ssq[:], AF.Sqrt, scale=1.0 / D,
                                     bias=eps_sb[:])
                rstd = stat.tile([128, 1], F32, tag="rstd")
                nc.vector.reciprocal(rstd[:], std[:])
                return rstd

            def transpose_to(dst_ap_3d, src_tile_ap, jlist):
                """PE-transpose 128x128 blocks into dst 3d view [128,len,128]."""
                ps = trps.tile([128, 512], BF16, tag="trp")
                for u, j in enumerate(jlist):
                    nc.tensor.transpose(
                        ps[:, u * 128:(u + 1) * 128],
                        src_tile_ap[:, j * 128:(j + 1) * 128],
                        id_sb[:],
                    )
                nc.vector.tensor_copy(
                    dst_ap_3d,
                    ps[:].rearrange("p (u x) -> p u x", u=len(jlist)),
                )

            # ---------------- phase 1: fc matmul ----------
            fps_tiles = {}

            def fc_group(tiles):
                """kk-outer over a group (used while wfc streams in)."""
                for key, _ in tiles:
                    fps_tiles[key] = mmps.tile([128, D], F32, tag="mm",
                                               name=f"fps{key}")
                for kk in range(NKK):
                    for key, st in tiles:
                        fps = fps_tiles[key]
                        for h in range(2):
                            nc.tensor.matmul(
                                fps[:, h * 512:(h + 1) * 512],
                                st[:, kk * 128:(kk + 1) * 128],
                                wfc_bf[:, kk * D + h * 512: kk * D + h * 512 + 512],
                                start=(kk == 0),
                                stop=(kk == NKK - 1),
                            )

            def fc_tile(key, st):
                """Tile-major full-rate fc for one 128-token tile."""
                fps = fps_tiles[key] = mmps.tile([128, D], F32, tag="mm",
                                                 name=f"fps{key}")
                for kk in range(NKK):
                    for h in range(2):
                        nc.tensor.matmul(
                            fps[:, h * 512:(h + 1) * 512],
                            st[:, kk * 128:(kk + 1) * 128],
                            wfc_bf[:, kk * D + h * 512: kk * D + h * 512 + 512],
                            start=(kk == 0),
                            stop=(kk == NKK - 1),
                        )

            def fused_epilogue(t):
                fps = fps_tiles[t]
                rstd = rms_stats(fps[:])
                fb = scr_bf.tile([128, D], BF16, tag="tmb")
                nc.vector.tensor_scalar(fb[:], fps[:], rstd[:], None, OP.mult)
                if apply_norm_weights:
                    nc.vector.tensor_tensor(fb[:], fb[:], whn_sb[:], op=OP.mult)
                bb, tl = divmod(t, 4)
                for g in range(2):
                    transpose_to(
                        fusedT_bv[bb][:, g * 4:(g + 1) * 4,
                                      tl * 128:(tl + 1) * 128],
                        fb[:],
                        [g * 4 + u for u in range(4)],
                    )

            def q_epilogue():
                fps = fps_tiles["t8"]
                rstd = rms_stats(fps[:])
                nc.vector.tensor_scalar(qs_f32[:], fps[:], rstd[:], None,
                                        OP.mult)
                if apply_norm_weights:
                    nc.vector.tensor_tensor(qs_f32[:], qs_f32[:], whn_sb[:],
                                            op=OP.mult)
                qhb = scr_bf.tile([128, D], BF16, tag="tmb")
                nc.vector.tensor_scalar(qhb[:], fps[:], rstd[:], None, OP.mult)
                if apply_norm_weights:
                    nc.vector.tensor_tensor(qhb[:], qhb[:], whn_sb[:],
                                            op=OP.mult)
                nc.vector.tensor_add(qhb[:], qhb[:], peq_bf[:])
                qht = scr_bf.tile([128, D], BF16, tag="tmb")
                qht_v = qht[:].rearrange("p (g x) -> p g x", g=2)
                for g in range(2):
                    transpose_to(
                        qht_v[:, g:g + 1, :].rearrange("p g x -> p (g x)")
                        .rearrange("p (u x) -> p u x", u=4),
                        qhb[:],
                        [g * 4 + u for u in range(4)],
                    )
                qps = mmps.tile([128, D], F32, tag="mm", name="qps")
                for j in range(NJ):
                    for h in range(2):
                        nc.tensor.matmul(
                            qps[:, h * 512:(h + 1) * 512],
                            qht[:, j * 128:(j + 1) * 128],
                            wq_sb[:, j * D + h * 512: j * D + h * 512 + 512],
                            start=(j == 0),
                            stop=(j == NJ - 1),
                        )
                qrstd = rms_stats(qps[:])
                qb = scr_bf.tile([128, D], BF16, tag="tmb")
                nc.vector.tensor_scalar(qb[:], qps[:], qrstd[:], None, OP.mult)
                if apply_norm_weights:
                    nc.vector.tensor_tensor(qb[:], qb[:], wqn_sb[:], op=OP.mult)
                nc.sync.dma_start(ag_in[:], qb[:])
                nc.gpsimd.collective_compute(
                    "AllGather", OP.bypass,
                    replica_groups=[list(range(NC))],
                    ins=[ag_in.opt()],
                    outs=[ag_out.opt()],
                )

            # -------- phase 2 helpers --------
            def k_tile(bb, tl):
                khb = scr_bf.tile([128, NJ * 128], BF16, tag="khb", bufs=2)
                nc.vector.tensor_add(
                    khb[:].rearrange("p (j x) -> p j x", j=NJ),
                    fusedT_bv[bb][:, :, tl * 128:(tl + 1) * 128],
                    pet_v[:, :, tl * 128:(tl + 1) * 128],
                )
                kps = mmps.tile([128, D], F32, tag="mm")
                for j in range(NJ):
                    for h in range(2):
                        nc.tensor.matmul(
                            kps[:, h * 512:(h + 1) * 512],
                            khb[:, j * 128:(j + 1) * 128],
                            wk_sb[:, j * D + h * 512: j * D + h * 512 + 512],
                            start=(j == 0),
                            stop=(j == NJ - 1),
                        )
                krstd = rms_stats(kps[:])
                kb = scr_bf.tile([128, D], BF16, tag="tmb")
                nc.vector.tensor_scalar(kb[:], kps[:], krstd[:], None, OP.mult)
                if apply_norm_weights:
                    nc.vector.tensor_tensor(kb[:], kb[:], wkn_sb[:], op=OP.mult)
                for g in range(2):
                    transpose_to(
                        kT_v[:, g * 4:(g + 1) * 4, tl * 128:(tl + 1) * 128],
                        kb[:],
                        [g * 4 + u for u in range(4)],
                    )

            def v_tile(bb, tl):
                vps = mmps.tile([128, D], F32, tag="mm")
                for j in range(NJ):
                    for h in range(2):
                        nc.tensor.matmul(
                            vps[:, h * 512:(h + 1) * 512],
                            fusedT_bv[bb][:, j:j + 1,
                                          tl * 128:(tl + 1) * 128]
                            .rearrange("p j x -> p (j x)"),
                            wv_sb[:, j * D + h * 512: j * D + h * 512 + 512],
                            start=(j == 0),
                            stop=(j == NJ - 1),
                        )
                nc.vector.tensor_copy(v_b[:, tl * D:(tl + 1) * D], vps[:])

            def q_tile_T(i):
                bb, k2 = divmod(i, 4)
                qg = scr_bf.tile([128, D], BF16, tag="tmb")
                r0 = (2 * k2) * 128 + bb * 64
                r1 = (2 * k2 + 1) * 128 + bb * 64
                nc.sync.dma_start(qg[0:64, :], ag_out[r0:r0 + 64, :])
                nc.sync.dma_start(qg[64:128, :], ag_out[r1:r1 + 64, :])
                qTt = scr_bf.tile([128, NJ * 128], BF16, tag="qTt", bufs=3,
                                  name=f"qT{i}")
                qTt_v = qTt[:].rearrange("p (j x) -> p j x", j=NJ)
                for g in range(2):
                    transpose_to(
                        qTt_v[:, g * 4:(g + 1) * 4, :],
                        qg[:],
                        [g * 4 + u for u in range(4)],
                    )
                return qTt

            def attn_tile(i, qTt, mask_ap, rs_buf, row0):
                sps = mmps.tile([128, CH], F32, tag="mm")
                for j in range(NJ):
                    nc.tensor.matmul(
                        sps[:],
                        qTt[:, j * 128:(j + 1) * 128],
                        kT[:, j * CH:(j + 1) * CH],
                        start=(j == 0),
                        stop=(j == NJ - 1),
                    )
                mb = scr_f.tile([128, CH], F32, tag="mb", bufs=2)
                nc.vector.tensor_scalar(mb[:], iota_sb[:],
                                        thr_sb[:, i:i + 1], MASK_NEG,
                                        OP.is_gt, OP.mult)
                sm = scr_f.tile([128, CH], F32, tag="mb", bufs=2)
                nc.vector.tensor_add(sm[:], sps[:], mb[:])
                o_sb = scr_f.tile([128, D + 1], F16, tag="osb", bufs=3)
                lacc = stat.tile([128, 1], F32, tag="lacc")
                probs = scr_bf.tile([128, CH], BF16, tag="probs", bufs=2)
                nc.scalar.activation(probs[:], sm[:], AF.Exp, scale=SCALE,
                                     bias=ebias_sb[:], accum_out=lacc[:])
                nc.vector.tensor_copy(o_sb[:, D:D + 1], lacc[:])
                pT = scr_bf.tile([128, 512], BF16, tag="pT", bufs=2)
                transpose_to(pT[:].rearrange("p (u x) -> p u x", u=4),
                             probs[:], list(range(4)))
                ops_ = mmps.tile([128, D], F32, tag="mm")
                for u in range(4):
                    for h in range(2):
                        nc.tensor.matmul(
                            ops_[:, h * 512:(h + 1) * 512],
                            pT[:, u * 128:(u + 1) * 128],
                            v_b[:, u * D + h * 512: u * D + h * 512 + 512],
                            start=(u == 0),
                            stop=(u == 3),
                        )
                nc.vector.tensor_copy(o_sb[:, 0:D], ops_[:])
                return nc.scalar.dma_start(rs_buf[row0:row0 + 128, :], o_sb[:])

            # ---------------- emission schedule ----------------
            # Group A streams with wfc; the rest is tile-major full-rate.
            fc_group([("t8", strip_q), (0, strips[0]), (1, strips[1])])
            q_epilogue()
            # strips t4..t7 queue behind the AllGather trigger
            for t in range(4, 8):
                strips[t] = load_strip(t, f"strip{t}")
            fused_epilogue(0)
            fused_epilogue(1)
            fc_group([(2, strips[2]), (3, strips[3])])
            fused_epilogue(2)
            fused_epilogue(3)

            for tl in range(4):
                k_tile(0, tl)
                v_tile(0, tl)

            for i in range(4):
                qTt = q_tile_T(i)
                attn_tile(i, qTt, None, rs_inA, i * 128)
            nc.gpsimd.collective_compute(
                "ReduceScatter", OP.add,
                replica_groups=[list(range(NC))],
                ins=[rs_inA.opt()],
                outs=[rs_outA.opt()],
            )

            fc_group([(4, strips[4]), (5, strips[5]), (6, strips[6])])
            fused_epilogue(4)
            fused_epilogue(5)
            fused_epilogue(6)
            fc_group([(7, strips[7])])
            fused_epilogue(7)
            for tl in range(4):
                k_tile(1, tl)
                v_tile(1, tl)
            for i in range(4, 8):
                qTt = q_tile_T(i)
                attn_tile(i, qTt, None, rs_inB, (i - 4) * 128)
            nc.gpsimd.collective_compute(
                "ReduceScatter", OP.add,
                replica_groups=[list(range(NC))],
                ins=[rs_inB.opt()],
                outs=[rs_outB.opt()],
            )

            # wo load (SWDGE, reuses wk's slot) — after the RS_B trigger so
            # its WAR wait doesn't block the collectives on the gpsimd queue.
            wo_sb = wpool.tile([128, NJ * D], BF16, tag="wko", name="wo_sb")
            nc.gpsimd.dma_start(
                wo_sb[:].rearrange("p (s c) -> p s c", s=NJ), sliced(wo))

            # ---------------- epilogue for own 128 queries --------------
            fo = scr_f.tile([QPC, D + 1], F16, tag="fo", bufs=1)
            nc.sync.dma_start(fo[0:64, :], rs_outA[:])
            nc.sync.dma_start(fo[64:128, :], rs_outB[:])
            linv = stat.tile([128, 1], F32, tag="linv")
            nc.vector.reciprocal(linv[:], fo[:, D:D + 1])
            ao = scr_bf.tile([128, D], BF16, tag="tmb")
            nc.vector.tensor_scalar(ao[:], fo[:, 0:D], linv[:], None, OP.mult)
            aoT = scr_bf.tile([128, D], BF16, tag="tmb")
            aoT_v = aoT[:].rearrange("p (g x) -> p g x", g=2)
            for g in range(2):
                transpose_to(
                    aoT_v[:, g:g + 1, :].rearrange("p g x -> p (g x)")
                    .rearrange("p (u x) -> p u x", u=4),
                    ao[:],
                    [g * 4 + u for u in range(4)],
                )
            zps = mmps.tile([128, D], F32, tag="mm")
            for j in range(NJ):
                for h in range(2):
                    nc.tensor.matmul(
                        zps[:, h * 512:(h + 1) * 512],
                        aoT[:, j * 128:(j + 1) * 128],
                        wo_sb[:, j * D + h * 512: j * D + h * 512 + 512],
                        start=(j == 0),
                        stop=(j == NJ - 1),
                    )
            hh = scr_f.tile([128, D], F32, tag="sqscr")
            nc.vector.tensor_add(hh[:], qs_f32[:], zps[:])
            orstd = rms_stats(hh[:])
            yv = scr_f.tile([128, D], F32, tag="sqscr")
            nc.vector.tensor_scalar(yv[:], hh[:], orstd[:], None, OP.mult)
            if apply_norm_weights:
                nc.vector.tensor_tensor(yv[:], yv[:], won_sb[:], op=OP.mult)
            nc.sync.dma_start(out[:], yv[:])

    nc.compile()
    return nc


def _pe_table():
    half = D // 2
    inv_freq = np.exp(np.arange(half, dtype=np.float32)
                      * (-math.log(10000.0) / half))
    ang = np.arange(S, dtype=np.float32)[:, None] * inv_freq
    return np.concatenate([np.sin(ang), np.cos(ang)], axis=-1).astype(np.float32)


def _core_gidx(c):
    """Global query indices owned by core c (64 batch-0 then 64 batch-1)."""
    h = QPC // 2
    return np.concatenate([np.arange(c * h, (c + 1) * h),
                           N + np.arange(c * h, (c + 1) * h)])


def make_in_maps(np_inputs, apply_w=False):
    hid = np.asarray(np_inputs["hidden_states"], np.float32)
    pos = np.asarray(np_inputs["context_positions"])
    Wfc = np.ascontiguousarray(np.asarray(np_inputs["W_fc"], np.float32))
    Wq = np.ascontiguousarray(np.asarray(np_inputs["Wq"], np.float32))
    Wk = np.ascontiguousarray(np.asarray(np_inputs["Wk"], np.float32))
    Wv = np.ascontiguousarray(np.asarray(np_inputs["Wv"], np.float32))
    Wo = np.ascontiguousarray(np.asarray(np_inputs["Wo"], np.float32))

    x = hid.reshape(B, S, LD)
    p = np.clip(pos.astype(np.int64), 0, S - 1)
    p_flat = p.reshape(QT)
    PE = _pe_table()

    iota_np = np.tile(np.arange(CH, dtype=np.float32), (128, 1))
    ident_np = np.eye(128, dtype=np.float32).astype(ml_dtypes.bfloat16)

    in_maps = []
    for c in range(NC):
        sl = slice(c * CH, (c + 1) * CH)
        gidx = _core_gidx(c)
        # strip layout [128 part, kk, 128 tok]: elem (p,kk,c) = x^T[kk*128+p, c]
        x3 = np.empty((NT, 128, NKK, 128), np.float32)
        xq_cols = x[gidx // N, p_flat[gidx]].T          # [3072, 128]
        x3[0] = xq_cols.reshape(NKK, 128, 128).transpose(1, 0, 2)
        for t in range(8):
            bb, tl = divmod(t, 4)
            r0 = c * CH + tl * 128
            x3[1 + t] = (x[bb, r0:r0 + 128, :].T
                         .reshape(NKK, 128, 128).transpose(1, 0, 2))
        peq_a = np.ascontiguousarray(PE[p_flat[gidx]])
        pet_a = np.ascontiguousarray(PE[sl].T)
        thr_a = np.ascontiguousarray(
            (p_flat.astype(np.float32) - c * CH).reshape(NC, 128).T)
        m = {
            "x3": x3.reshape(NT * 128, NKK * 128),
            "wfc": Wfc, "wq": Wq, "wk": Wk, "wv": Wv, "wo": Wo,
            "pet": pet_a, "peq": peq_a, "thr": thr_a,
            "iota": iota_np, "ident": ident_np,
        }
        if apply_w:
            m["whn"] = np.tile(np.asarray(np_inputs["w_hidden_norm"], np.float32), (128, 1))
            m["wqn"] = np.tile(np.asarray(np_inputs["w_q_norm"], np.float32), (128, 1))
            m["wkn"] = np.tile(np.asarray(np_inputs["w_k_norm"], np.float32), (128, 1))
            m["won"] = np.tile(np.asarray(np_inputs["w_out_norm"], np.float32), (128, 1))
        in_maps.append(m)
    return in_maps


def assemble_out(results):
    y = np.zeros((QT, D), np.float32)
    for c in range(NC):
        y[_core_gidx(c)] = results[c]["out"]
    return y.reshape(B, N, D)


def kernel(**inputs) -> np.ndarray:
    w_h = np.asarray(inputs["w_hidden_norm"], np.float32)
    w_q = np.asarray(inputs["w_q_norm"], np.float32)
    w_k = np.asarray(inputs["w_k_norm"], np.float32)
    w_o = np.asarray(inputs["w_out_norm"], np.float32)
    apply_w = not (np.all(w_h == 1) and np.all(w_q == 1)
                   and np.all(w_k == 1) and np.all(w_o == 1))

    key = ("nc", apply_w)
    if key not in _CACHE:
        _CACHE[key] = _build(apply_w)
    nc = _CACHE[key]

    in_maps = make_in_maps(inputs, apply_w)

    trace = os.environ.get("KERNEL_TRACE", "0") == "1"
    if trace:
        try:
            import axon_prof
            axon_prof.install()
        except Exception:
            trace = False
    res = run_bass_kernel_spmd(nc, in_maps, list(range(NC)), trace=trace)
    global LAST_EXEC_NS
    LAST_EXEC_NS = res.exec_time_ns

    return assemble_out(res.results).astype(np.float32)


LAST_EXEC_NS = None
